# revision 1
# baseline (speedup 1.0000x reference)
"""GCN (3-layer GCNConv + global mean pool) on 8 Trainium2 NeuronCores.

Math: with S = adjacency+self-loops and D = diag(1/sqrt(deg)),
    conv(h) = relu(D S D h W + b)
and the diagonal scalings commute with the dense W, so each layer is an
UNWEIGHTED gather-sum of pre-scaled rows plus a dense matmul.  The final
conv + mean-pool collapse into a dense host-precomputed pooling matrix:
    out = (Mhat @ h2) @ W2 + b2,   Mhat = diag(1/cnt) S_pool A_norm.

Sharding: nodes dst-partitioned across 8 cores; layer 0 is recomputed on
each core's halo (src nodes of its incident edges) so cores never
communicate.  Gathers use the GPSIMD dma_gather custom instruction
(int16 indices -> sources split into <=32767-row windows; gather columns
of 128 rows feed identity matmuls that accumulate transposed sums in
PSUM).  Per-node 1/sqrt(deg) scaling is fused into the ReLU on ScalarE.
Lane assignment sorts nodes by their per-window in-edge count vector to
minimize column padding; all index arithmetic happens on the host, and
the 8 cores share one program (shared column-count profiles).
"""

import hashlib

import numpy as np
import ml_dtypes

P = 128
NCORES = 8
WSTR = 32768      # physical window stride (rows); row w*WSTR is all-zero
WIN0 = 32767      # usable rows per layer-0 source window
WIN1 = 32256      # usable slots per layer-1 source window (512-aligned)
GCOLS = 96        # gather column budget per tile-group
CCAP = 32         # max columns per dma_gather call

BF16 = ml_dtypes.bfloat16


# ---------------------------------------------------------------------------
# Shared schedule derivation (host + builder + emulator all use this)
# ---------------------------------------------------------------------------

def _schedule(D, NW):
    """D: [T][NW] column counts.  Returns (groups, calls, colbase, Ctot).

    groups: list of (t0, t1) tile ranges with total columns <= GCOLS.
    calls: list of (w, ncols, col_off) in global column order; a call's
        columns are consecutive.  Global column order: per group, per
        window (ascending), per tile (ascending), per j.
    colbase: [T][NW] global column offset of (t, w)'s first column.
    """
    T = len(D)
    groups = []
    t = 0
    while t < T:
        tot = sum(D[t])
        t1 = t + 1
        while t1 < T and tot + sum(D[t1]) <= GCOLS:
            tot += sum(D[t1])
            t1 += 1
        groups.append((t, t1))
        t = t1
    colbase = [[0] * NW for _ in range(T)]
    calls = []
    off = 0
    for (t0, t1) in groups:
        for w in range(NW):
            cols = 0
            for tt in range(t0, t1):
                colbase[tt][w] = off + cols
                cols += D[tt][w]
            c0 = 0
            while c0 < cols:
                n = min(CCAP, cols - c0)
                calls.append((w, n, off + c0))
                c0 += n
            off += cols
    return groups, calls, colbase, off


def _call_of_col(calls):
    """Map global column -> (call_id, local_col)."""
    m = {}
    for ci, (w, n, off) in enumerate(calls):
        for j in range(n):
            m[off + j] = (ci, j)
    return m


# ---------------------------------------------------------------------------
# Host preprocessing
# ---------------------------------------------------------------------------

def _edge_expand(nodes, degi, s_sorted, indptr):
    """Expand in-edge lists (self-loop last) for `nodes`.
    Returns (slot_rep, jj, srcs): for each edge, owning node position in
    `nodes`, edge rank, and src global id."""
    dg = degi[nodes]
    tot = int(dg.sum())
    rep = np.repeat(np.arange(len(nodes)), dg)
    jj = np.arange(tot) - np.repeat(np.cumsum(dg) - dg, dg)
    g = nodes[rep]
    is_self = jj == (dg[rep] - 1)
    ei = np.minimum(indptr[g] + jj, len(s_sorted) - 1)
    srcs = np.where(is_self, g, s_sorted[ei])
    return rep, jj, srcs


def _assign_slots(nodes, wvec, T):
    """Sort nodes by per-window count vector (lexsort, first window most
    significant last => np.lexsort(wvec.T)), pads first."""
    order = np.lexsort(wvec.T)
    slot_node = np.full(T * P, -1, np.int64)
    slot_node[T * P - len(nodes):] = nodes[order]
    return slot_node


def _dprof_w(slot_node, node_wvec_lookup, T, NW):
    """Per-tile per-window max counts. node_wvec_lookup: [n_universe, NW]."""
    vec = np.zeros((T * P, NW), np.int64)
    v = slot_node >= 0
    vec[v] = node_wvec_lookup[slot_node[v]]
    return vec.reshape(T, P, NW).max(axis=1)


def _fill_idx(slot_node, D, colbase, Ctot, degi, s_sorted, indptr,
              src_key, win):
    """Build the flat int16 gather index list [Ctot*128] (0 = window zero
    row), then wrap for dma_gather: [128, Ctot*8]."""
    flat = np.zeros(Ctot * P, np.int16)
    k = np.nonzero(slot_node >= 0)[0]
    nodes = slot_node[k]
    rep, jj, srcs = _edge_expand(nodes, degi, s_sorted, indptr)
    keys = src_key[srcs]
    w_e = keys // win
    loc = keys % win + 1
    slot = k[rep]
    tt = slot // P
    pp = slot % P
    # rank within (edge's slot, window): edges of one node are contiguous in
    # rep order; stable sort by window within each node run.
    o = np.lexsort((jj, w_e, rep))
    so_rep, so_w = rep[o], w_e[o]
    grp_change = np.ones(len(o), bool)
    grp_change[1:] = (so_rep[1:] != so_rep[:-1]) | (so_w[1:] != so_w[:-1])
    gid = np.cumsum(grp_change) - 1
    starts = np.nonzero(grp_change)[0]
    rank_sorted = np.arange(len(o)) - starts[gid]
    rank = np.empty(len(o), np.int64)
    rank[o] = rank_sorted
    cb = np.asarray(colbase, np.int64)
    col = cb[tt, w_e] + rank
    flat[col * P + pp] = loc.astype(np.int16)
    wrapped = np.tile(flat.reshape(-1, 16).T, (8, 1))
    return flat, wrapped


def _preprocess(x, edge_index, batch, num_graphs, W0, b0, W1, b1, W2, b2):
    x = np.asarray(x, np.float32)
    N, IN = x.shape
    HID = W0.shape[1]
    G = int(num_graphs)
    SH = N // NCORES
    src = np.asarray(edge_index[0], np.int64)
    dst = np.asarray(edge_index[1], np.int64)
    batch = np.asarray(batch, np.int64)

    degi = np.bincount(dst, minlength=N) + 1          # + self-loop
    dis = (1.0 / np.sqrt(degi.astype(np.float64))).astype(np.float32)
    invd = np.sqrt(degi.astype(np.float64)).astype(np.float32)

    order = np.argsort(dst, kind="stable")
    s_sorted = src[order]
    indptr = np.searchsorted(dst, np.arange(N + 1), sorter=order)

    # layer-0 windowed source (shared): row w*WSTR+1+i = dis[g]*x[g], g=w*WIN0+i
    NW0 = -(-N // WIN0)
    xh_g = np.zeros((NW0 * WSTR, IN), np.float32)
    xhat = x * dis[:, None]
    for w in range(NW0):
        g0, g1 = w * WIN0, min((w + 1) * WIN0, N)
        xh_g[w * WSTR + 1: w * WSTR + 1 + (g1 - g0)] = xhat[g0:g1]
    xh_g = xh_g.astype(BF16)

    # dense pooling matrix Mhat = diag(1/cnt) @ S_pool @ A_norm  [G, N]
    cnt = np.bincount(batch, minlength=G).astype(np.float64)
    cntc = np.maximum(cnt, 1.0)
    bd = batch[dst]
    w_ = dis[dst].astype(np.float64) * dis[src] / cntc[bd]
    M = np.bincount(bd * N + src, weights=w_, minlength=G * N)
    w2_ = dis.astype(np.float64) ** 2 / cntc[batch]
    M += np.bincount(batch * N + np.arange(N), weights=w2_, minlength=G * N)
    Mhat = M.reshape(G, N).astype(np.float32)

    # per-node layer-0 window count vectors (global: window of src node id)
    rep_all, _, srcs_all = _edge_expand(np.arange(N), degi, s_sorted, indptr)
    wvec0_all = np.zeros((N, NW0), np.int64)
    np.add.at(wvec0_all, (rep_all, srcs_all // WIN0), 1)

    halos, owns = [], []
    for c in range(NCORES):
        own = np.arange(c * SH, (c + 1) * SH)
        halo = np.unique(np.concatenate(
            [s_sorted[indptr[c * SH]:indptr[(c + 1) * SH]], own]))
        owns.append(own)
        halos.append(halo)

    T0 = max(-(-len(h) // P) for h in halos)
    T0 = -(-T0 // 4) * 4
    T1 = -(-(-(-SH // P)) // 4) * 4
    assert T0 * P <= 2 * WIN1, "layer-1 source exceeds two windows"
    NW1 = -(-(T0 * P) // WIN1)

    # Per-pattern slot budgeting: every distinct window-count vector gets
    # max-over-cores slots at a FIXED position, so all cores' tile profiles
    # coincide and the cross-core max adds (almost) no padding.
    uniqpat, pat_of = np.unique(wvec0_all, axis=0, return_inverse=True)
    npat = len(uniqpat)
    pcounts = np.zeros((NCORES, npat), np.int64)
    for c in range(NCORES):
        pcounts[c] = np.bincount(pat_of[halos[c]], minlength=npat)
    budget = pcounts.max(axis=0)
    pstart = np.concatenate([[0], np.cumsum(budget)])
    T0 = -(-int(pstart[-1]) // P)
    T0 = -(-T0 // 4) * 4

    def _assign_budgeted(nodes):
        po = pat_of[nodes]
        order = np.argsort(po, kind="stable")
        pos = np.searchsorted(po[order], np.arange(npat))
        rank = np.arange(len(nodes)) - pos[po[order]]
        slot_node = np.full(T0 * P, -1, np.int64)
        slot_node[pstart[po[order]] + rank] = nodes[order]
        return slot_node

    slot0 = [_assign_budgeted(halos[c]) for c in range(NCORES)]
    assert T0 * P <= 2 * WIN1, "layer-1 source exceeds two windows"
    NW1 = -(-(T0 * P) // WIN1)
    D0 = np.max([_dprof_w(s, wvec0_all, T0, NW0) for s in slot0], axis=0)

    # layer-1 window vectors depend on each core's own slot0 (src slot ids)
    pos0s, wvec1s = [], []
    for c in range(NCORES):
        pos0 = np.full(N, -1, np.int64)
        v = slot0[c] >= 0
        pos0[slot0[c][v]] = np.nonzero(v)[0]
        pos0s.append(pos0)
        rep, _, srcs = _edge_expand(owns[c], degi, s_sorted, indptr)
        wv = np.zeros((SH, NW1), np.int64)
        np.add.at(wv, (rep, pos0[srcs] // WIN1), 1)
        wvec1s.append(wv)
    slot1 = [_assign_slots(owns[c], wvec1s[c], T1) for c in range(NCORES)]
    wvec1_full = []
    for c in range(NCORES):
        full = np.zeros((N, NW1), np.int64)
        full[owns[c]] = wvec1s[c]
        wvec1_full.append(full)
    D1 = np.max([_dprof_w(slot1[c], wvec1_full[c], T1, NW1)
                 for c in range(NCORES)], axis=0)

    D0t = tuple(tuple(int(v) for v in row) for row in D0)
    D1t = tuple(tuple(int(v) for v in row) for row in D1)
    _, _, colbase0, C0 = _schedule(D0t, NW0)
    _, _, colbase1, C1 = _schedule(D1t, NW1)

    ident0 = np.arange(N, dtype=np.int64)
    cores = []
    for c in range(NCORES):
        s0, s1 = slot0[c], slot1[c]
        v0, v1 = s0 >= 0, s1 >= 0
        flat0, idx0 = _fill_idx(s0, D0t, colbase0, C0, degi, s_sorted, indptr,
                                ident0, WIN0)
        flat1, idx1 = _fill_idx(s1, D1t, colbase1, C1, degi, s_sorted, indptr,
                                pos0s[c], WIN1)

        def _scales(s, v, T, vec):
            d = np.where(v, vec[np.clip(s, 0, None)], 0).astype(np.float32)
            iv = np.where(v, invd[np.clip(s, 0, None)], 0).astype(np.float32)
            return d.reshape(T, P).T.copy(), iv.reshape(1, T * P)

        # layer-0 output is stored pre-scaled for the next gather:
        # h1_hat = dis*relu(dis*(u@W0)+b0) = relu(dis^2*(u@W0)+dis*b0)
        dis0, inv0 = _scales(s0, v0, T0, dis * dis)
        dis1, inv1 = _scales(s1, v1, T1, dis)
        mct = np.zeros((T1 * P, G), np.float32)
        mct[v1] = Mhat[:, s1[v1]].T
        cores.append(dict(idx0=idx0, idx1=idx1, dis0=dis0, dis1=dis1,
                          inv0=inv0.astype(BF16), inv1=inv1.astype(BF16),
                          mct=mct.astype(BF16), flat0=flat0, flat1=flat1))

    shared = dict(
        xh=xh_g,
        w0=np.ascontiguousarray(W0, np.float32).astype(BF16),
        w1=np.ascontiguousarray(W1, np.float32).reshape(2, P, HID).astype(BF16),
        b0r=np.ascontiguousarray(b0, np.float32).reshape(1, HID).astype(BF16),
        b1r=np.ascontiguousarray(b1, np.float32).reshape(1, HID).astype(BF16),
        ident=np.eye(P, dtype=np.float32).astype(BF16),
    )
    zero_bias = bool(np.all(np.asarray(b0) == 0) and np.all(np.asarray(b1) == 0))
    meta = dict(N=N, IN=IN, HID=HID, G=G, SH=SH, T0=T0, T1=T1,
                NW0=NW0, NW1=NW1, C0=C0, C1=C1, zero_bias=zero_bias,
                D0=D0t, D1=D1t)
    fin = dict(W2=np.asarray(W2, np.float32), b2=np.asarray(b2, np.float32))
    return meta, shared, cores, fin


# ---------------------------------------------------------------------------
# Pure-numpy emulation of the device program (validation / debugging)
# ---------------------------------------------------------------------------

def _emulate(meta, shared, cores, fin):
    T0, T1, HID, IN, G = (meta[k] for k in ("T0", "T1", "HID", "IN", "G"))
    NW0, NW1 = meta["NW0"], meta["NW1"]
    D0, D1 = meta["D0"], meta["D1"]
    _, calls0, colbase0, C0 = _schedule(D0, NW0)
    _, calls1, colbase1, C1 = _schedule(D1, NW1)
    xh = shared["xh"].astype(np.float32)
    w0 = shared["w0"].astype(np.float32)
    w1 = shared["w1"].astype(np.float32).reshape(2 * P, HID)
    b0 = shared["b0r"].astype(np.float32)[0]
    b1 = shared["b1r"].astype(np.float32)[0]

    def col_windows(calls, C):
        cw = np.zeros(C, np.int64)
        for w, n, off in calls:
            cw[off:off + n] = w
        return cw

    cw0 = col_windows(calls0, C0)
    cw1 = col_windows(calls1, C1)

    Y = np.zeros((G, HID), np.float32)
    for cd in cores:
        rows = cw0.repeat(P) * WSTR + cd["flat0"]
        gat = xh[rows].reshape(C0, P, IN)
        u0 = np.zeros((T0 * P, IN), np.float32)
        for t in range(T0):
            for w in range(NW0):
                cb = colbase0[t][w]
                for j in range(D0[t][w]):
                    u0[t * P:(t + 1) * P] += gat[cb + j]
        u0 = u0.astype(BF16).astype(np.float32)
        inv0 = cd["inv0"].astype(np.float32)[0]
        pre = u0 @ w0 + inv0[:, None] * b0[None, :]
        d0 = cd["dis0"].T.reshape(-1)
        h1 = np.maximum(d0[:, None] * pre, 0).astype(BF16).astype(np.float32)
        # place h1 into windowed layout
        h1w = np.zeros((NW1 * WSTR, HID), np.float32)
        for w in range(NW1):
            sl0, sl1 = w * WIN1, min((w + 1) * WIN1, T0 * P)
            h1w[w * WSTR + 1: w * WSTR + 1 + (sl1 - sl0)] = h1[sl0:sl1]

        rows1 = cw1.repeat(P) * WSTR + cd["flat1"]
        gat1 = h1w[rows1].reshape(C1, P, HID)
        u1 = np.zeros((T1 * P, HID), np.float32)
        for t in range(T1):
            for w in range(NW1):
                cb = colbase1[t][w]
                for j in range(D1[t][w]):
                    u1[t * P:(t + 1) * P] += gat1[cb + j]
        u1 = u1.astype(BF16).astype(np.float32)
        inv1 = cd["inv1"].astype(np.float32)[0]
        pre1 = u1 @ w1 + inv1[:, None] * b1[None, :]
        d1 = cd["dis1"].T.reshape(-1)
        h2 = np.maximum(d1[:, None] * pre1, 0).astype(BF16).astype(np.float32)
        Y += cd["mct"].astype(np.float32).T @ h2
    return Y @ fin["W2"] + fin["b2"]


# ---------------------------------------------------------------------------
# Bass device program
# ---------------------------------------------------------------------------

def _build(meta):
    import concourse.bass as bass
    import concourse.mybir as mybir
    import concourse.tile as tile
    from concourse import bacc, library_config
    from concourse.tile_rust import add_dep_helper

    F32, I16 = mybir.dt.float32, mybir.dt.int16
    BF = mybir.dt.bfloat16
    RELU = mybir.ActivationFunctionType.Relu

    IN, HID, G = meta["IN"], meta["HID"], meta["G"]
    T0, T1 = meta["T0"], meta["T1"]
    NW0, NW1 = meta["NW0"], meta["NW1"]
    C0, C1 = meta["C0"], meta["C1"]
    D0, D1 = meta["D0"], meta["D1"]
    ZB = meta["zero_bias"]
    groups0, calls0, colbase0, _ = _schedule(D0, NW0)
    groups1, calls1, colbase1, _ = _schedule(D1, NW1)
    c2c0 = _call_of_col(calls0)
    c2c1 = _call_of_col(calls1)

    nc = bacc.Bacc("TRN2", target_bir_lowering=False, debug=False,
                   num_devices=NCORES)

    t_xh = nc.dram_tensor("xh", [NW0 * WSTR, IN], BF, kind="ExternalInput")
    t_idx0 = nc.dram_tensor("idx0", [P, C0 * 8], I16, kind="ExternalInput")
    t_idx1 = nc.dram_tensor("idx1", [P, C1 * 8], I16, kind="ExternalInput")
    t_dis0 = nc.dram_tensor("dis0", [P, T0], F32, kind="ExternalInput")
    t_dis1 = nc.dram_tensor("dis1", [P, T1], F32, kind="ExternalInput")
    t_inv0 = nc.dram_tensor("inv0", [1, T0 * P], BF, kind="ExternalInput")
    t_inv1 = nc.dram_tensor("inv1", [1, T1 * P], BF, kind="ExternalInput")
    t_w0 = nc.dram_tensor("w0", [IN, HID], BF, kind="ExternalInput")
    t_w1 = nc.dram_tensor("w1", [2, P, HID], BF, kind="ExternalInput")
    t_b0 = nc.dram_tensor("b0r", [1, HID], BF, kind="ExternalInput")
    t_b1 = nc.dram_tensor("b1r", [1, HID], BF, kind="ExternalInput")
    t_mct = nc.dram_tensor("mct", [T1 * P, G], BF, kind="ExternalInput")
    t_id = nc.dram_tensor("ident", [P, P], BF, kind="ExternalInput")
    t_out = nc.dram_tensor("outp", [G, HID], F32, kind="ExternalOutput")

    with tile.TileContext(nc) as tc:
        with (
            tc.tile_pool(name="const", bufs=1) as cpool,
            tc.tile_pool(name="ut", bufs=6) as upool,
            tc.tile_pool(name="stage", bufs=3) as spool,
            tc.tile_pool(name="aggps", bufs=4, space="PSUM") as apool,
            tc.tile_pool(name="preps", bufs=2, space="PSUM") as ppool,
            tc.tile_pool(name="outps", bufs=1, space="PSUM") as opool,
            tc.tile_pool(name="dram", bufs=1, space="DRAM") as dpool,
        ):
            lib = nc.gpsimd.load_library(library_config.mlp)

            def cload(t, shape, dt):
                s = cpool.tile(shape, dt, tag=t.name)
                nc.sync.dma_start(s[:], t[:])
                return s

            ident = cload(t_id, [P, P], BF)
            w0 = cload(t_w0, [IN, HID], BF)
            w1 = cpool.tile([P, 2, HID], BF, tag="w1")
            nc.sync.dma_start(w1[:], t_w1[:].rearrange("j p h -> p j h"))
            if not ZB:
                b0r = cload(t_b0, [1, HID], BF)
                b1r = cload(t_b1, [1, HID], BF)
            dis0 = cload(t_dis0, [P, T0], F32)
            dis1 = cload(t_dis1, [P, T1], F32)

            h1h = dpool.tile([NW1 * WSTR, HID], BF)
            h2d = dpool.tile([T1 * P, HID], BF)

            h1h_writes = []
            zt = spool.tile([P, HID], BF, tag="zrow")
            nc.vector.memset(zt[:], 0.0)
            for w in range(NW1):
                h1h_writes.append(nc.sync.dma_start(
                    h1h[w * WSTR:w * WSTR + 1, :], zt[0:1, :]))

            def slot_row(s):
                return (s // WIN1) * WSTR + 1 + (s % WIN1)

            def do_gathers(t_src_ap, idx_sb, calls, grp_calls, buf_pool,
                           ccap_elem, elem, deps):
                out = {}
                for ci in grp_calls:
                    w, ncols, off = calls[ci]
                    gt = buf_pool.tile([P, ccap_elem], BF, tag="g")
                    gi = nc.gpsimd.dma_gather(
                        gt[:, :ncols * elem].rearrange(
                            "p (j d) -> p j d", j=ncols),
                        t_src_ap(w),
                        idx_sb[:, off * 8:(off + ncols) * 8],
                        ncols * P, ncols * P, elem, single_packet=False)
                    add_dep_helper(gi.ins, lib.ins, True, "gather after lib")
                    for d in deps:
                        add_dep_helper(gi.ins, d.ins, True, "gather after src")
                    out[ci] = gt
                return out

            def grp_call_ids(calls, t0, t1, colbase, D, NW):
                ids = set()
                for tt in range(t0, t1):
                    for w in range(NW):
                        for j in range(D[tt][w]):
                            ids.add(_CALLMAP[colbase[tt][w] + j][0])
                return sorted(ids)

            # ---------------- Layer 0 ----------------
            with tc.tile_pool(name="idx0p", bufs=1) as ipool0, \
                 tc.tile_pool(name="g0", bufs=8) as gpool0:
                idx0 = ipool0.tile([P, C0 * 8], I16, tag="idx0")
                nc.sync.dma_start(idx0[:], t_idx0[:])

                global _CALLMAP
                _CALLMAP = c2c0
                stage_t = None
                for (t0g, t1g) in groups0:
                    ids = grp_call_ids(calls0, t0g, t1g, colbase0, D0, NW0)
                    bufs = do_gathers(
                        lambda w: t_xh[w * WSTR:(w + 1) * WSTR, :],
                        idx0, calls0, ids, gpool0, CCAP * IN, IN, [])
                    for t in range(t0g, t1g):
                        nd = sum(D0[t])
                        ups = apool.tile([P, P], mybir.dt.float32,
                                         tag="aggps", space="PSUM")
                        k = 0
                        for w in range(NW0):
                            for j in range(D0[t][w]):
                                ci, lc = c2c0[colbase0[t][w] + j]
                                gt = bufs[ci]
                                nc.tensor.matmul(
                                    ups[:], lhsT=gt[:, lc * IN:(lc + 1) * IN],
                                    rhs=ident[:], start=(k == 0),
                                    stop=(k == nd - 1))
                                k += 1
                        uT = upool.tile([P, P], BF, tag="ut")
                        if nd == 0:
                            nc.vector.memset(uT[:], 0.0)
                        else:
                            nc.vector.tensor_copy(uT[:], ups[:])
                        pre = ppool.tile([P, HID], mybir.dt.float32,
                                         tag="preps", space="PSUM")
                        if ZB:
                            nc.tensor.matmul(pre[:], lhsT=uT[:], rhs=w0[:],
                                             start=True, stop=True)
                        else:
                            sd = upool.tile([1, P], BF, tag="seed")
                            nc.sync.dma_start(sd[:],
                                              t_inv0[:, t * P:(t + 1) * P])
                            nc.tensor.matmul(pre[:], lhsT=sd[:], rhs=b0r[:],
                                             start=True, stop=False)
                            nc.tensor.matmul(pre[:], lhsT=uT[:], rhs=w0[:],
                                             start=False, stop=True)
                        sl = t % 4
                        if sl == 0:
                            stage_t = spool.tile([P, 4 * HID], BF,
                                                 tag="h1stage")
                        nc.scalar.activation(
                            stage_t[:, sl * HID:(sl + 1) * HID], pre[:],
                            RELU, bias=0.0, scale=dis0[:, t:t + 1])
                        if sl == 3:
                            r0 = slot_row((t - 3) * P)
                            h1h_writes.append(nc.sync.dma_start(
                                h1h[r0:r0 + 4 * P, :].rearrange(
                                    "(j p) h -> p j h", p=P),
                                stage_t[:].rearrange("p (j h) -> p j h", j=4)))

            # ---------------- Layer 1 ----------------
            with tc.tile_pool(name="idx1p", bufs=1) as ipool1, \
                 tc.tile_pool(name="g1", bufs=6) as gpool1:
                idx1 = ipool1.tile([P, C1 * 8], I16, tag="idx1")
                nc.sync.dma_start(idx1[:], t_idx1[:])

                _CALLMAP = c2c1
                h2_writes = []
                stage_t = None
                for (t0g, t1g) in groups1:
                    ids = grp_call_ids(calls1, t0g, t1g, colbase1, D1, NW1)
                    bufs = do_gathers(
                        lambda w: h1h[w * WSTR:(w + 1) * WSTR, :],
                        idx1, calls1, ids, gpool1, CCAP * HID, HID,
                        h1h_writes)
                    for t in range(t0g, t1g):
                        nd = sum(D1[t])
                        u0ps = apool.tile([P, P], mybir.dt.float32,
                                          tag="aggps", space="PSUM")
                        u1ps = apool.tile([P, P], mybir.dt.float32,
                                          tag="aggps", space="PSUM")
                        k = 0
                        for w in range(NW1):
                            for j in range(D1[t][w]):
                                ci, lc = c2c1[colbase1[t][w] + j]
                                gt = bufs[ci]
                                nc.tensor.matmul(
                                    u0ps[:],
                                    lhsT=gt[:, lc * HID:lc * HID + P],
                                    rhs=ident[:], start=(k == 0),
                                    stop=(k == nd - 1))
                                nc.tensor.matmul(
                                    u1ps[:],
                                    lhsT=gt[:, lc * HID + P:(lc + 1) * HID],
                                    rhs=ident[:], start=(k == 0),
                                    stop=(k == nd - 1))
                                k += 1
                        uT0 = upool.tile([P, P], BF, tag="ut")
                        uT1 = upool.tile([P, P], BF, tag="ut")
                        if nd == 0:
                            nc.vector.memset(uT0[:], 0.0)
                            nc.vector.memset(uT1[:], 0.0)
                        else:
                            nc.vector.tensor_copy(uT0[:], u0ps[:])
                            nc.vector.tensor_copy(uT1[:], u1ps[:])
                        pre = ppool.tile([P, HID], mybir.dt.float32,
                                         tag="preps", space="PSUM")
                        if not ZB:
                            sd = upool.tile([1, P], BF, tag="seed")
                            nc.sync.dma_start(sd[:],
                                              t_inv1[:, t * P:(t + 1) * P])
                            nc.tensor.matmul(pre[:], lhsT=sd[:], rhs=b1r[:],
                                             start=True, stop=False)
                        nc.tensor.matmul(pre[:], lhsT=uT0[:], rhs=w1[:, 0, :],
                                         start=ZB, stop=False)
                        nc.tensor.matmul(pre[:], lhsT=uT1[:], rhs=w1[:, 1, :],
                                         start=False, stop=True)
                        sl = t % 4
                        if sl == 0:
                            stage_t = spool.tile([P, 4 * HID], BF,
                                                 tag="h1stage")
                        nc.scalar.activation(
                            stage_t[:, sl * HID:(sl + 1) * HID], pre[:],
                            RELU, bias=0.0, scale=dis1[:, t:t + 1])
                        if sl == 3:
                            h2_writes.append(nc.sync.dma_start(
                                h2d[(t - 3) * P:(t + 1) * P, :].rearrange(
                                    "(j p) h -> p j h", p=P),
                                stage_t[:].rearrange("p (j h) -> p j h", j=4)))

            # ---------------- Layer 2 + pool ----------------
            with tc.tile_pool(name="mc", bufs=3) as mpool, \
                 tc.tile_pool(name="h2s", bufs=3) as h2pool:
                ops = opool.tile([G, HID], mybir.dt.float32, tag="outps",
                                 space="PSUM")
                for tb in range(0, T1, 4):
                    mt = mpool.tile([P, 4 * G], BF, tag="mc")
                    nc.sync.dma_start(
                        mt[:].rearrange("p (j g) -> p j g", j=4),
                        t_mct[tb * P:(tb + 4) * P, :].rearrange(
                            "(j p) g -> p j g", p=P))
                    ht = h2pool.tile([P, 4 * HID], BF, tag="h2s")
                    hw = nc.sync.dma_start(
                        ht[:].rearrange("p (j h) -> p j h", j=4),
                        h2d[tb * P:(tb + 4) * P, :].rearrange(
                            "(j p) h -> p j h", p=P))
                    add_dep_helper(hw.ins, h2_writes[tb // 4].ins, True,
                                   "h2 readback after write")
                    for q in range(4):
                        t = tb + q
                        nc.tensor.matmul(
                            ops[:], lhsT=mt[:, q * G:(q + 1) * G],
                            rhs=ht[:, q * HID:(q + 1) * HID],
                            start=(t == 0), stop=(t == T1 - 1))
                osb = spool.tile([G, HID], mybir.dt.float32, tag="osb")
                nc.vector.tensor_copy(osb[:], ops[:])
                nc.sync.dma_start(t_out[:], osb[:])

    nc.compile()
    return nc


# ---------------------------------------------------------------------------
# Entry point
# ---------------------------------------------------------------------------

_cache = {}


def _get_nc(meta):
    key = hashlib.sha1(repr(sorted(meta.items())).encode()).hexdigest()
    if key not in _cache:
        _cache[key] = _build(meta)
    return _cache[key]


def _in_maps(shared, cores):
    maps = []
    for cd in cores:
        m = dict(shared)
        m.update({k: cd[k] for k in
                  ("idx0", "idx1", "dis0", "dis1", "inv0", "inv1", "mct")})
        maps.append(m)
    return maps


def _run_device(meta, shared, cores):
    from concourse.bass_utils import run_bass_kernel_spmd
    nc = _get_nc(meta)
    res = run_bass_kernel_spmd(nc, _in_maps(shared, cores),
                               core_ids=list(range(NCORES)))
    return [r["outp"] for r in res.results]


def kernel(**inputs):
    meta, shared, cores, fin = _preprocess(**inputs)
    outs = _run_device(meta, shared, cores)
    Y = np.sum(np.stack(outs), axis=0, dtype=np.float32)
    out = Y @ fin["W2"] + fin["b2"]
    return out.astype(np.float32)


def profile_run(meta, shared, cores, trace_cores=None):
    """Profiled exec time in ns: NTFF trace when available, else the
    instruction-cost-model timeline simulation of the compiled program."""
    from concourse.bass_utils import run_bass_kernel_spmd
    nc = _get_nc(meta)
    try:
        res = run_bass_kernel_spmd(nc, _in_maps(shared, cores),
                                   core_ids=list(range(NCORES)), trace=True,
                                   trace_cores=trace_cores)
        if res.exec_time_ns is not None:
            print("profile:", res.instructions_and_trace[1]
                  if res.instructions_and_trace else None)
            return res.exec_time_ns
    except Exception as e:
        print(f"NTFF trace unavailable ({type(e).__name__}); "
              "using cost-model timeline")
    from concourse.timeline_sim import TimelineSim
    ts = TimelineSim(nc, trace=False)
    ts.simulate()
    return int(ts.time)



# revision 2
# speedup vs baseline: 1.5472x; 1.5472x over previous
"""GCN (3-layer GCNConv + global mean pool) on 8 Trainium2 NeuronCores.

Math: with S = adjacency+self-loops and D = diag(1/sqrt(deg)),
    conv(h) = relu(D S D h W + b)
and the diagonal scalings commute with the dense W, so each layer is an
UNWEIGHTED gather-sum of pre-scaled rows plus a dense matmul.  The final
conv + mean-pool collapse into a dense host-precomputed pooling matrix:
    out = (Mhat @ h2) @ W2 + b2,   Mhat = diag(1/cnt) S_pool A_norm.

Sharding: nodes dst-partitioned across 8 cores; layer 0 is recomputed on
each core's halo (src nodes of its incident edges) so cores never
communicate.  Layer 0 reads a host-prebuilt edge-expanded message table
(one x̂ row per in-edge, laid out column-major per dst tile) with plain
full-bandwidth contiguous DMA; per-tile transpose-matmuls with an
identity then accumulate the message sums in PSUM.  Layer 1 gathers h1
rows with the GPSIMD dma_gather custom instruction (int16 indices ->
sources split into <=32767-row windows).  Per-node 1/sqrt(deg) scaling
is fused into the ReLU on ScalarE.  Halo nodes are degree-sorted so the
8 cores can share one program (shared per-tile column counts) with
minimal padding; all index arithmetic happens on the host.
"""

import hashlib

import numpy as np
import ml_dtypes

P = 128
NCORES = 8
WSTR = 32768      # physical window stride (rows); row w*WSTR is all-zero
WIN1 = 32256      # usable slots per layer-1 source window (512-aligned)
GC0 = 128         # layer-0 table columns per DMA chunk
GCOLS = 96        # layer-1 gather column budget per tile-group
CCAP = 32         # max columns per dma_gather call

BF16 = ml_dtypes.bfloat16


# ---------------------------------------------------------------------------
# Shared schedule derivation (host + builder + emulator all use this)
# ---------------------------------------------------------------------------

def _schedule(D, NW):
    """D: [T][NW] column counts.  Returns (groups, calls, colbase, Ctot).

    groups: list of (t0, t1) tile ranges with total columns <= GCOLS.
    calls: list of (w, ncols, col_off) in global column order; a call's
        columns are consecutive.  Global column order: per group, per
        window (ascending), per tile (ascending), per j.
    colbase: [T][NW] global column offset of (t, w)'s first column.
    """
    T = len(D)
    groups = []
    t = 0
    while t < T:
        tot = sum(D[t])
        t1 = t + 1
        while t1 < T and tot + sum(D[t1]) <= GCOLS:
            tot += sum(D[t1])
            t1 += 1
        groups.append((t, t1))
        t = t1
    colbase = [[0] * NW for _ in range(T)]
    calls = []
    off = 0
    for (t0, t1) in groups:
        for w in range(NW):
            cols = 0
            for tt in range(t0, t1):
                colbase[tt][w] = off + cols
                cols += D[tt][w]
            c0 = 0
            while c0 < cols:
                n = min(CCAP, cols - c0)
                calls.append((w, n, off + c0))
                c0 += n
            off += cols
    return groups, calls, colbase, off


def _schedule0(D0):
    """Layer-0 chunking: greedy tile groups with <= GC0 columns each.
    Returns (groups, colbase) with colbase[t] the global column offset."""
    T = len(D0)
    groups = []
    t = 0
    while t < T:
        tot = D0[t]
        t1 = t + 1
        while t1 < T and tot + D0[t1] <= GC0:
            tot += D0[t1]
            t1 += 1
        groups.append((t, t1))
        t = t1
    colbase = [0] * (T + 1)
    for t in range(T):
        colbase[t + 1] = colbase[t] + D0[t]
    return groups, colbase


def _call_of_col(calls):
    """Map global column -> (call_id, local_col)."""
    m = {}
    for ci, (w, n, off) in enumerate(calls):
        for j in range(n):
            m[off + j] = (ci, j)
    return m


# ---------------------------------------------------------------------------
# Host preprocessing
# ---------------------------------------------------------------------------

def _edge_expand(nodes, degi, s_sorted, indptr):
    """Expand in-edge lists (self-loop last) for `nodes`.
    Returns (slot_rep, jj, srcs): for each edge, owning node position in
    `nodes`, edge rank, and src global id."""
    dg = degi[nodes]
    tot = int(dg.sum())
    rep = np.repeat(np.arange(len(nodes)), dg)
    jj = np.arange(tot) - np.repeat(np.cumsum(dg) - dg, dg)
    g = nodes[rep]
    is_self = jj == (dg[rep] - 1)
    ei = np.minimum(indptr[g] + jj, len(s_sorted) - 1)
    srcs = np.where(is_self, g, s_sorted[ei])
    return rep, jj, srcs


def _assign_slots(nodes, wvec, T):
    """Sort nodes by per-window count vector (lexsort, first window most
    significant last => np.lexsort(wvec.T)), pads first."""
    order = np.lexsort(wvec.T)
    slot_node = np.full(T * P, -1, np.int64)
    slot_node[T * P - len(nodes):] = nodes[order]
    return slot_node


def _dprof_w(slot_node, node_wvec_lookup, T, NW):
    """Per-tile per-window max counts. node_wvec_lookup: [n_universe, NW]."""
    vec = np.zeros((T * P, NW), np.int64)
    v = slot_node >= 0
    vec[v] = node_wvec_lookup[slot_node[v]]
    return vec.reshape(T, P, NW).max(axis=1)


def _fill_idx(slot_node, D, colbase, Ctot, degi, s_sorted, indptr,
              src_key, win):
    """Build the flat int16 gather index list [Ctot*128] (0 = window zero
    row), then wrap for dma_gather: [128, Ctot*8]."""
    flat = np.zeros(Ctot * P, np.int16)
    k = np.nonzero(slot_node >= 0)[0]
    nodes = slot_node[k]
    rep, jj, srcs = _edge_expand(nodes, degi, s_sorted, indptr)
    keys = src_key[srcs]
    w_e = keys // win
    loc = keys % win + 1
    slot = k[rep]
    tt = slot // P
    pp = slot % P
    # rank within (edge's slot, window): edges of one node are contiguous in
    # rep order; stable sort by window within each node run.
    o = np.lexsort((jj, w_e, rep))
    so_rep, so_w = rep[o], w_e[o]
    grp_change = np.ones(len(o), bool)
    grp_change[1:] = (so_rep[1:] != so_rep[:-1]) | (so_w[1:] != so_w[:-1])
    gid = np.cumsum(grp_change) - 1
    starts = np.nonzero(grp_change)[0]
    rank_sorted = np.arange(len(o)) - starts[gid]
    rank = np.empty(len(o), np.int64)
    rank[o] = rank_sorted
    cb = np.asarray(colbase, np.int64)
    col = cb[tt, w_e] + rank
    flat[col * P + pp] = loc.astype(np.int16)
    wrapped = np.tile(flat.reshape(-1, 16).T, (8, 1))
    return flat, wrapped


def _preprocess(x, edge_index, batch, num_graphs, W0, b0, W1, b1, W2, b2):
    x = np.asarray(x, np.float32)
    N, IN = x.shape
    HID = W0.shape[1]
    G = int(num_graphs)
    SH = N // NCORES
    src = np.asarray(edge_index[0], np.int64)
    dst = np.asarray(edge_index[1], np.int64)
    batch = np.asarray(batch, np.int64)

    degi = np.bincount(dst, minlength=N) + 1          # + self-loop
    dis = (1.0 / np.sqrt(degi.astype(np.float64))).astype(np.float32)
    invd = np.sqrt(degi.astype(np.float64)).astype(np.float32)

    order = np.argsort(dst, kind="stable")
    s_sorted = src[order]
    indptr = np.searchsorted(dst, np.arange(N + 1), sorter=order)

    xhat = (x * dis[:, None]).astype(np.float32)

    # dense pooling matrix Mhat = diag(1/cnt) @ S_pool @ A_norm  [G, N]
    cnt = np.bincount(batch, minlength=G).astype(np.float64)
    cntc = np.maximum(cnt, 1.0)
    bd = batch[dst]
    w_ = dis[dst].astype(np.float64) * dis[src] / cntc[bd]
    M = np.bincount(bd * N + src, weights=w_, minlength=G * N)
    w2_ = dis.astype(np.float64) ** 2 / cntc[batch]
    M += np.bincount(batch * N + np.arange(N), weights=w2_, minlength=G * N)
    Mhat = M.reshape(G, N).astype(np.float32)

    halos, owns = [], []
    for c in range(NCORES):
        own = np.arange(c * SH, (c + 1) * SH)
        halo = np.unique(np.concatenate(
            [s_sorted[indptr[c * SH]:indptr[(c + 1) * SH]], own]))
        owns.append(own)
        halos.append(halo)

    # Layer 0: degree-sorted halo slots so all cores share one per-tile
    # column profile D0 with minimal padding.
    T0 = max(-(-len(h) // P) for h in halos)
    T0 = -(-T0 // 4) * 4
    assert T0 * P <= 2 * WIN1, "layer-1 source exceeds two windows"
    NW1 = -(-(T0 * P) // WIN1)
    T1 = -(-(-(-SH // P)) // 4) * 4

    slot0 = []
    for c in range(NCORES):
        h = halos[c]
        o = np.argsort(-degi[h], kind="stable")
        sn = np.full(T0 * P, -1, np.int64)
        sn[:len(h)] = h[o]
        slot0.append(sn)
    degmat = np.zeros((NCORES, T0 * P), np.int64)
    for c in range(NCORES):
        v = slot0[c] >= 0
        degmat[c][v] = degi[slot0[c][v]]
    D0 = degmat.reshape(NCORES, T0, P).max(axis=(0, 2))  # [T0]
    D0t = tuple(int(v) for v in D0)
    groups0, colbase0 = _schedule0(D0t)
    C0 = colbase0[T0]

    # layer-1 window vectors depend on each core's own slot0 (src slot ids)
    pos0s, wvec1s = [], []
    for c in range(NCORES):
        pos0 = np.full(N, -1, np.int64)
        v = slot0[c] >= 0
        pos0[slot0[c][v]] = np.nonzero(v)[0]
        pos0s.append(pos0)
        rep, _, srcs = _edge_expand(owns[c], degi, s_sorted, indptr)
        wv = np.zeros((SH, NW1), np.int64)
        np.add.at(wv, (rep, pos0[srcs] // WIN1), 1)
        wvec1s.append(wv)
    slot1 = [_assign_slots(owns[c], wvec1s[c], T1) for c in range(NCORES)]
    wvec1_full = []
    for c in range(NCORES):
        full = np.zeros((N, NW1), np.int64)
        full[owns[c]] = wvec1s[c]
        wvec1_full.append(full)
    D1 = np.max([_dprof_w(slot1[c], wvec1_full[c], T1, NW1)
                 for c in range(NCORES)], axis=0)
    D1t = tuple(tuple(int(v) for v in row) for row in D1)
    _, _, colbase1, C1 = _schedule(D1t, NW1)

    cb0 = np.asarray(colbase0[:T0], np.int64)
    cores = []
    for c in range(NCORES):
        s0, s1 = slot0[c], slot1[c]
        v0, v1 = s0 >= 0, s1 >= 0

        # layer-0 edge-expanded message table [P, C0*IN]
        X0 = np.zeros((C0, P, IN), np.float32)
        k = np.nonzero(v0)[0]
        nodes = s0[k]
        rep, jj, srcs = _edge_expand(nodes, degi, s_sorted, indptr)
        slot = k[rep]
        col = cb0[slot // P] + jj
        X0[col, slot % P] = xhat[srcs]
        x0 = np.ascontiguousarray(
            X0.transpose(1, 0, 2).reshape(P, C0 * IN)).astype(BF16)

        flat1, idx1 = _fill_idx(s1, D1t, colbase1, C1, degi, s_sorted, indptr,
                                pos0s[c], WIN1)

        def _scales(s, v, T, vec):
            d = np.where(v, vec[np.clip(s, 0, None)], 0).astype(np.float32)
            iv = np.where(v, invd[np.clip(s, 0, None)], 0).astype(np.float32)
            return d.reshape(T, P).T.copy(), iv.reshape(1, T * P)

        # layer-0 output is stored pre-scaled for the next gather:
        # h1_hat = dis*relu(dis*(u@W0)+b0) = relu(dis^2*(u@W0)+dis*b0)
        dis0, inv0 = _scales(s0, v0, T0, dis * dis)
        dis1, inv1 = _scales(s1, v1, T1, dis)
        mct = np.zeros((T1 * P, G), np.float32)
        mct[v1] = Mhat[:, s1[v1]].T
        cores.append(dict(x0=x0, idx1=idx1, dis0=dis0, dis1=dis1,
                          inv0=inv0.astype(BF16), inv1=inv1.astype(BF16),
                          mct=mct.astype(BF16), flat1=flat1))

    shared = dict(
        w0=np.ascontiguousarray(W0, np.float32).astype(BF16),
        w1=np.ascontiguousarray(W1, np.float32).reshape(2, P, HID).astype(BF16),
        b0r=np.ascontiguousarray(b0, np.float32).reshape(1, HID).astype(BF16),
        b1r=np.ascontiguousarray(b1, np.float32).reshape(1, HID).astype(BF16),
        ident=np.eye(P, dtype=np.float32).astype(BF16),
    )
    zero_bias = bool(np.all(np.asarray(b0) == 0) and np.all(np.asarray(b1) == 0))
    meta = dict(N=N, IN=IN, HID=HID, G=G, SH=SH, T0=T0, T1=T1,
                NW1=NW1, C0=C0, C1=C1, zero_bias=zero_bias,
                D0=D0t, D1=D1t)
    fin = dict(W2=np.asarray(W2, np.float32), b2=np.asarray(b2, np.float32))
    return meta, shared, cores, fin


# ---------------------------------------------------------------------------
# Pure-numpy emulation of the device program (validation / debugging)
# ---------------------------------------------------------------------------

def _emulate(meta, shared, cores, fin):
    T0, T1, HID, IN, G = (meta[k] for k in ("T0", "T1", "HID", "IN", "G"))
    NW1 = meta["NW1"]
    D0, D1 = meta["D0"], meta["D1"]
    _, colbase0 = _schedule0(D0)
    _, calls1, colbase1, C1 = _schedule(D1, NW1)
    w0 = shared["w0"].astype(np.float32)
    w1 = shared["w1"].astype(np.float32).reshape(2 * P, HID)
    b0 = shared["b0r"].astype(np.float32)[0]
    b1 = shared["b1r"].astype(np.float32)[0]

    def col_windows(calls, C):
        cw = np.zeros(C, np.int64)
        for w, n, off in calls:
            cw[off:off + n] = w
        return cw

    cw1 = col_windows(calls1, C1)

    Y = np.zeros((G, HID), np.float32)
    for cd in cores:
        tab = cd["x0"].astype(np.float32).reshape(P, -1, IN).transpose(1, 0, 2)
        u0 = np.zeros((T0 * P, IN), np.float32)
        for t in range(T0):
            for j in range(D0[t]):
                u0[t * P:(t + 1) * P] += tab[colbase0[t] + j]
        u0 = u0.astype(BF16).astype(np.float32)
        inv0 = cd["inv0"].astype(np.float32)[0]
        pre = u0 @ w0 + inv0[:, None] * b0[None, :]
        d0 = cd["dis0"].T.reshape(-1)
        h1 = np.maximum(d0[:, None] * pre, 0).astype(BF16).astype(np.float32)
        # place h1 into windowed layout
        h1w = np.zeros((NW1 * WSTR, HID), np.float32)
        for w in range(NW1):
            sl0, sl1 = w * WIN1, min((w + 1) * WIN1, T0 * P)
            h1w[w * WSTR + 1: w * WSTR + 1 + (sl1 - sl0)] = h1[sl0:sl1]

        rows1 = cw1.repeat(P) * WSTR + cd["flat1"]
        gat1 = h1w[rows1].reshape(C1, P, HID)
        u1 = np.zeros((T1 * P, HID), np.float32)
        for t in range(T1):
            for w in range(NW1):
                cb = colbase1[t][w]
                for j in range(D1[t][w]):
                    u1[t * P:(t + 1) * P] += gat1[cb + j]
        u1 = u1.astype(BF16).astype(np.float32)
        inv1 = cd["inv1"].astype(np.float32)[0]
        pre1 = u1 @ w1 + inv1[:, None] * b1[None, :]
        d1 = cd["dis1"].T.reshape(-1)
        h2 = np.maximum(d1[:, None] * pre1, 0).astype(BF16).astype(np.float32)
        Y += cd["mct"].astype(np.float32).T @ h2
    return Y @ fin["W2"] + fin["b2"]


# ---------------------------------------------------------------------------
# Bass device program
# ---------------------------------------------------------------------------

def _build(meta):
    import concourse.bass as bass
    import concourse.mybir as mybir
    import concourse.tile as tile
    from concourse import bacc, library_config
    from concourse.tile_rust import add_dep_helper

    F32, I16 = mybir.dt.float32, mybir.dt.int16
    BF = mybir.dt.bfloat16
    RELU = mybir.ActivationFunctionType.Relu

    IN, HID, G = meta["IN"], meta["HID"], meta["G"]
    T0, T1 = meta["T0"], meta["T1"]
    NW1 = meta["NW1"]
    C0, C1 = meta["C0"], meta["C1"]
    D0, D1 = meta["D0"], meta["D1"]
    ZB = meta["zero_bias"]
    groups0, colbase0 = _schedule0(D0)
    groups1, calls1, colbase1, _ = _schedule(D1, NW1)
    c2c1 = _call_of_col(calls1)

    nc = bacc.Bacc("TRN2", target_bir_lowering=False, debug=False,
                   num_devices=NCORES)

    t_x0 = nc.dram_tensor("x0", [P, C0 * IN], BF, kind="ExternalInput")
    t_idx1 = nc.dram_tensor("idx1", [P, C1 * 8], I16, kind="ExternalInput")
    t_dis0 = nc.dram_tensor("dis0", [P, T0], F32, kind="ExternalInput")
    t_dis1 = nc.dram_tensor("dis1", [P, T1], F32, kind="ExternalInput")
    t_inv0 = nc.dram_tensor("inv0", [1, T0 * P], BF, kind="ExternalInput")
    t_inv1 = nc.dram_tensor("inv1", [1, T1 * P], BF, kind="ExternalInput")
    t_w0 = nc.dram_tensor("w0", [IN, HID], BF, kind="ExternalInput")
    t_w1 = nc.dram_tensor("w1", [2, P, HID], BF, kind="ExternalInput")
    t_b0 = nc.dram_tensor("b0r", [1, HID], BF, kind="ExternalInput")
    t_b1 = nc.dram_tensor("b1r", [1, HID], BF, kind="ExternalInput")
    t_mct = nc.dram_tensor("mct", [T1 * P, G], BF, kind="ExternalInput")
    t_id = nc.dram_tensor("ident", [P, P], BF, kind="ExternalInput")
    t_out = nc.dram_tensor("outp", [G, HID], F32, kind="ExternalOutput")

    with tile.TileContext(nc) as tc:
        with (
            tc.tile_pool(name="const", bufs=1) as cpool,
            tc.tile_pool(name="ut", bufs=6) as upool,
            tc.tile_pool(name="stage", bufs=3) as spool,
            tc.tile_pool(name="aggps", bufs=4, space="PSUM") as apool,
            tc.tile_pool(name="preps", bufs=2, space="PSUM") as ppool,
            tc.tile_pool(name="outps", bufs=1, space="PSUM") as opool,
            tc.tile_pool(name="dram", bufs=1, space="DRAM") as dpool,
        ):
            lib = nc.gpsimd.load_library(library_config.mlp)

            def cload(t, shape, dt):
                s = cpool.tile(shape, dt, tag=t.name)
                nc.sync.dma_start(s[:], t[:])
                return s

            ident = cload(t_id, [P, P], BF)
            w0 = cload(t_w0, [IN, HID], BF)
            w1 = cpool.tile([P, 2, HID], BF, tag="w1")
            nc.sync.dma_start(w1[:], t_w1[:].rearrange("j p h -> p j h"))
            if not ZB:
                b0r = cload(t_b0, [1, HID], BF)
                b1r = cload(t_b1, [1, HID], BF)
            dis0 = cload(t_dis0, [P, T0], F32)
            dis1 = cload(t_dis1, [P, T1], F32)

            h1h = dpool.tile([NW1 * WSTR, HID], BF)
            h2d = dpool.tile([T1 * P, HID], BF)

            h1h_writes = []
            zt = spool.tile([P, HID], BF, tag="zrow")
            nc.vector.memset(zt[:], 0.0)
            for w in range(NW1):
                h1h_writes.append(nc.sync.dma_start(
                    h1h[w * WSTR:w * WSTR + 1, :], zt[0:1, :]))

            def slot_row(s):
                return (s // WIN1) * WSTR + 1 + (s % WIN1)

            # ---------------- Layer 0 ----------------
            with tc.tile_pool(name="x0p", bufs=3) as xpool:
                stage_t = None
                for (t0g, t1g) in groups0:
                    off = colbase0[t0g]
                    ncol = colbase0[t1g] - off
                    xt = xpool.tile([P, GC0 * IN], BF, tag="x0")
                    nc.sync.dma_start(xt[:, :ncol * IN],
                                      t_x0[:, off * IN:(off + ncol) * IN])
                    for t in range(t0g, t1g):
                        nd = D0[t]
                        base = colbase0[t] - off
                        ups = apool.tile([P, P], mybir.dt.float32,
                                         tag="aggps", space="PSUM")
                        for j in range(nd):
                            nc.tensor.matmul(
                                ups[:],
                                lhsT=xt[:, (base + j) * IN:(base + j + 1) * IN],
                                rhs=ident[:], start=(j == 0),
                                stop=(j == nd - 1))
                        uT = upool.tile([P, P], BF, tag="ut")
                        if nd == 0:
                            nc.vector.memset(uT[:], 0.0)
                        else:
                            nc.vector.tensor_copy(uT[:], ups[:])
                        pre = ppool.tile([P, HID], mybir.dt.float32,
                                         tag="preps", space="PSUM")
                        if ZB:
                            nc.tensor.matmul(pre[:], lhsT=uT[:], rhs=w0[:],
                                             start=True, stop=True)
                        else:
                            sd = upool.tile([1, P], BF, tag="seed")
                            nc.sync.dma_start(sd[:],
                                              t_inv0[:, t * P:(t + 1) * P])
                            nc.tensor.matmul(pre[:], lhsT=sd[:], rhs=b0r[:],
                                             start=True, stop=False)
                            nc.tensor.matmul(pre[:], lhsT=uT[:], rhs=w0[:],
                                             start=False, stop=True)
                        sl = t % 4
                        if sl == 0:
                            stage_t = spool.tile([P, 4 * HID], BF,
                                                 tag="h1stage")
                        nc.scalar.activation(
                            stage_t[:, sl * HID:(sl + 1) * HID], pre[:],
                            RELU, bias=0.0, scale=dis0[:, t:t + 1])
                        if sl == 3:
                            r0 = slot_row((t - 3) * P)
                            h1h_writes.append(nc.sync.dma_start(
                                h1h[r0:r0 + 4 * P, :].rearrange(
                                    "(j p) h -> p j h", p=P),
                                stage_t[:].rearrange("p (j h) -> p j h", j=4)))

            # ---------------- Layer 1 ----------------
            with tc.tile_pool(name="idx1p", bufs=1) as ipool1, \
                 tc.tile_pool(name="g1", bufs=6) as gpool1:
                idx1 = ipool1.tile([P, C1 * 8], I16, tag="idx1")
                nc.sync.dma_start(idx1[:], t_idx1[:])

                def do_gathers(t_src_ap, idx_sb, calls, grp_calls, buf_pool,
                               ccap_elem, elem, deps):
                    out = {}
                    for ci in grp_calls:
                        w, ncols, off = calls[ci]
                        gt = buf_pool.tile([P, ccap_elem], BF, tag="g")
                        gi = nc.gpsimd.dma_gather(
                            gt[:, :ncols * elem].rearrange(
                                "p (j d) -> p j d", j=ncols),
                            t_src_ap(w),
                            idx_sb[:, off * 8:(off + ncols) * 8],
                            ncols * P, ncols * P, elem, single_packet=False)
                        add_dep_helper(gi.ins, lib.ins, True,
                                       "gather after lib")
                        for d in deps:
                            add_dep_helper(gi.ins, d.ins, True,
                                           "gather after src")
                        out[ci] = gt
                    return out

                def grp_call_ids(calls, t0, t1, colbase, D, NW):
                    ids = set()
                    for tt in range(t0, t1):
                        for w in range(NW):
                            for j in range(D[tt][w]):
                                ids.add(c2c1[colbase[tt][w] + j][0])
                    return sorted(ids)

                h2_writes = []
                stage_t = None
                for (t0g, t1g) in groups1:
                    ids = grp_call_ids(calls1, t0g, t1g, colbase1, D1, NW1)
                    bufs = do_gathers(
                        lambda w: h1h[w * WSTR:(w + 1) * WSTR, :],
                        idx1, calls1, ids, gpool1, CCAP * HID, HID,
                        h1h_writes)
                    for t in range(t0g, t1g):
                        nd = sum(D1[t])
                        u0ps = apool.tile([P, P], mybir.dt.float32,
                                          tag="aggps", space="PSUM")
                        u1ps = apool.tile([P, P], mybir.dt.float32,
                                          tag="aggps", space="PSUM")
                        k = 0
                        for w in range(NW1):
                            for j in range(D1[t][w]):
                                ci, lc = c2c1[colbase1[t][w] + j]
                                gt = bufs[ci]
                                nc.tensor.matmul(
                                    u0ps[:],
                                    lhsT=gt[:, lc * HID:lc * HID + P],
                                    rhs=ident[:], start=(k == 0),
                                    stop=(k == nd - 1))
                                nc.tensor.matmul(
                                    u1ps[:],
                                    lhsT=gt[:, lc * HID + P:(lc + 1) * HID],
                                    rhs=ident[:], start=(k == 0),
                                    stop=(k == nd - 1))
                                k += 1
                        uT0 = upool.tile([P, P], BF, tag="ut")
                        uT1 = upool.tile([P, P], BF, tag="ut")
                        if nd == 0:
                            nc.vector.memset(uT0[:], 0.0)
                            nc.vector.memset(uT1[:], 0.0)
                        else:
                            nc.vector.tensor_copy(uT0[:], u0ps[:])
                            nc.vector.tensor_copy(uT1[:], u1ps[:])
                        pre = ppool.tile([P, HID], mybir.dt.float32,
                                         tag="preps", space="PSUM")
                        if not ZB:
                            sd = upool.tile([1, P], BF, tag="seed")
                            nc.sync.dma_start(sd[:],
                                              t_inv1[:, t * P:(t + 1) * P])
                            nc.tensor.matmul(pre[:], lhsT=sd[:], rhs=b1r[:],
                                             start=True, stop=False)
                        nc.tensor.matmul(pre[:], lhsT=uT0[:], rhs=w1[:, 0, :],
                                         start=ZB, stop=False)
                        nc.tensor.matmul(pre[:], lhsT=uT1[:], rhs=w1[:, 1, :],
                                         start=False, stop=True)
                        sl = t % 4
                        if sl == 0:
                            stage_t = spool.tile([P, 4 * HID], BF,
                                                 tag="h1stage")
                        nc.scalar.activation(
                            stage_t[:, sl * HID:(sl + 1) * HID], pre[:],
                            RELU, bias=0.0, scale=dis1[:, t:t + 1])
                        if sl == 3:
                            h2_writes.append(nc.sync.dma_start(
                                h2d[(t - 3) * P:(t + 1) * P, :].rearrange(
                                    "(j p) h -> p j h", p=P),
                                stage_t[:].rearrange("p (j h) -> p j h", j=4)))

            # ---------------- Layer 2 + pool ----------------
            with tc.tile_pool(name="mc", bufs=3) as mpool, \
                 tc.tile_pool(name="h2s", bufs=3) as h2pool:
                ops = opool.tile([G, HID], mybir.dt.float32, tag="outps",
                                 space="PSUM")
                for tb in range(0, T1, 4):
                    mt = mpool.tile([P, 4 * G], BF, tag="mc")
                    nc.sync.dma_start(
                        mt[:].rearrange("p (j g) -> p j g", j=4),
                        t_mct[tb * P:(tb + 4) * P, :].rearrange(
                            "(j p) g -> p j g", p=P))
                    ht = h2pool.tile([P, 4 * HID], BF, tag="h2s")
                    hw = nc.sync.dma_start(
                        ht[:].rearrange("p (j h) -> p j h", j=4),
                        h2d[tb * P:(tb + 4) * P, :].rearrange(
                            "(j p) h -> p j h", p=P))
                    add_dep_helper(hw.ins, h2_writes[tb // 4].ins, True,
                                   "h2 readback after write")
                    for q in range(4):
                        t = tb + q
                        nc.tensor.matmul(
                            ops[:], lhsT=mt[:, q * G:(q + 1) * G],
                            rhs=ht[:, q * HID:(q + 1) * HID],
                            start=(t == 0), stop=(t == T1 - 1))
                osb = spool.tile([G, HID], mybir.dt.float32, tag="osb")
                nc.vector.tensor_copy(osb[:], ops[:])
                nc.sync.dma_start(t_out[:], osb[:])

    nc.compile()
    return nc


# ---------------------------------------------------------------------------
# Entry point
# ---------------------------------------------------------------------------

_cache = {}


def _get_nc(meta):
    key = hashlib.sha1(repr(sorted(meta.items())).encode()).hexdigest()
    if key not in _cache:
        _cache[key] = _build(meta)
    return _cache[key]


def _in_maps(shared, cores):
    maps = []
    for cd in cores:
        m = dict(shared)
        m.update({k: cd[k] for k in
                  ("x0", "idx1", "dis0", "dis1", "inv0", "inv1", "mct")})
        maps.append(m)
    return maps


def _run_device(meta, shared, cores):
    from concourse.bass_utils import run_bass_kernel_spmd
    nc = _get_nc(meta)
    res = run_bass_kernel_spmd(nc, _in_maps(shared, cores),
                               core_ids=list(range(NCORES)))
    return [r["outp"] for r in res.results]


def kernel(**inputs):
    meta, shared, cores, fin = _preprocess(**inputs)
    outs = _run_device(meta, shared, cores)
    Y = np.sum(np.stack(outs), axis=0, dtype=np.float32)
    out = Y @ fin["W2"] + fin["b2"]
    return out.astype(np.float32)


def profile_run(meta, shared, cores, trace_cores=None):
    """Profiled exec time in ns: NTFF trace when available, else the
    instruction-cost-model timeline simulation of the compiled program."""
    from concourse.bass_utils import run_bass_kernel_spmd
    nc = _get_nc(meta)
    try:
        res = run_bass_kernel_spmd(nc, _in_maps(shared, cores),
                                   core_ids=list(range(NCORES)), trace=True,
                                   trace_cores=trace_cores)
        if res.exec_time_ns is not None:
            print("profile:", res.instructions_and_trace[1]
                  if res.instructions_and_trace else None)
            return res.exec_time_ns
    except Exception as e:
        print(f"NTFF trace unavailable ({type(e).__name__}); "
              "using cost-model timeline")
    from concourse.timeline_sim import TimelineSim
    ts = TimelineSim(nc, trace=False)
    ts.simulate()
    return int(ts.time)


# revision 14
# speedup vs baseline: 1.9242x; 1.2437x over previous
"""GCN (3-layer GCNConv + global mean pool) on 8 Trainium2 NeuronCores.

Math: with S = adjacency+self-loops and D = diag(1/sqrt(deg)),
    conv(h) = relu(D S D h W + b)
and the diagonal scalings commute with the dense W, so each layer is an
UNWEIGHTED gather-sum of pre-scaled rows plus a dense matmul.  The final
conv + mean-pool collapse into a dense host-precomputed pooling matrix:
    out = (Mhat @ h2) @ W2 + b2,   Mhat = diag(1/cnt) S_pool A_norm.

Sharding: nodes dst-partitioned across 8 cores; layer 0 is recomputed on
each core's halo (src nodes of its incident edges) so cores never
communicate.  Layer 0 reads a host-prebuilt edge-expanded message table
(one x̂ row per in-edge, laid out column-major per dst tile) with plain
full-bandwidth contiguous DMA; per-tile transpose-matmuls with an
identity then accumulate the message sums in PSUM.  Layer 1 gathers h1
rows with the GPSIMD dma_gather custom instruction (int16 indices ->
sources split into <=32767-row windows).  Per-node 1/sqrt(deg) scaling
is fused into the ReLU on ScalarE.  Halo nodes are degree-sorted so the
8 cores can share one program (shared per-tile column counts) with
minimal padding; all index arithmetic happens on the host.
"""

import hashlib

import numpy as np
import ml_dtypes

P = 128
NCORES = 8
WSTR = 32768      # physical window stride (rows); row w*WSTR is all-zero
WIN1 = 32256      # usable slots per layer-1 source window (512-aligned)
GC0 = 128         # layer-0 table columns per DMA chunk
GCOLS = 96        # layer-1 gather column budget per tile-group
CCAP = 32         # max columns per dma_gather call

BF16 = ml_dtypes.bfloat16
F8 = ml_dtypes.float8_e4m3    # TRN FP8_EXP4-compatible for |v| <= 240


# ---------------------------------------------------------------------------
# Shared schedule derivation (host + builder + emulator all use this)
# ---------------------------------------------------------------------------

def _schedule(D, NW):
    """D: [T][NW] column counts.  Returns (groups, calls, colbase, Ctot).

    groups: list of (t0, t1) tile ranges with total columns <= GCOLS.
    calls: list of (w, ncols, col_off) in global column order; a call's
        columns are consecutive.  Global column order: per group, per
        window (ascending), per tile (ascending), per j.
    colbase: [T][NW] global column offset of (t, w)'s first column.
    """
    T = len(D)
    groups = []
    t = 0
    while t < T:
        tot = sum(D[t])
        t1 = t + 1
        while t1 < T and tot + sum(D[t1]) <= GCOLS:
            tot += sum(D[t1])
            t1 += 1
        groups.append((t, t1))
        t = t1
    colbase = [[0] * NW for _ in range(T)]
    calls = []
    off = 0
    for (t0, t1) in groups:
        for w in range(NW):
            cols = 0
            for tt in range(t0, t1):
                colbase[tt][w] = off + cols
                cols += D[tt][w]
            c0 = 0
            while c0 < cols:
                n = min(CCAP, cols - c0)
                calls.append((w, n, off + c0))
                c0 += n
            off += cols
    return groups, calls, colbase, off


def _schedule0(D0):
    """Layer-0 chunking: greedy tile groups with <= GC0 columns each.
    Returns (groups, colbase) with colbase[t] the global column offset."""
    T = len(D0)
    groups = []
    t = 0
    while t < T:
        tot = D0[t]
        t1 = t + 1
        while t1 < T and tot + D0[t1] <= GC0:
            tot += D0[t1]
            t1 += 1
        groups.append((t, t1))
        t = t1
    colbase = [0] * (T + 1)
    for t in range(T):
        colbase[t + 1] = colbase[t] + D0[t]
    return groups, colbase


def _call_of_col(calls):
    """Map global column -> (call_id, local_col)."""
    m = {}
    for ci, (w, n, off) in enumerate(calls):
        for j in range(n):
            m[off + j] = (ci, j)
    return m


# ---------------------------------------------------------------------------
# Host preprocessing
# ---------------------------------------------------------------------------

def _edge_expand(nodes, degi, s_sorted, indptr):
    """Expand in-edge lists (self-loop last) for `nodes`.
    Returns (slot_rep, jj, srcs): for each edge, owning node position in
    `nodes`, edge rank, and src global id."""
    dg = degi[nodes]
    tot = int(dg.sum())
    rep = np.repeat(np.arange(len(nodes)), dg)
    jj = np.arange(tot) - np.repeat(np.cumsum(dg) - dg, dg)
    g = nodes[rep]
    is_self = jj == (dg[rep] - 1)
    ei = np.minimum(indptr[g] + jj, len(s_sorted) - 1)
    srcs = np.where(is_self, g, s_sorted[ei])
    return rep, jj, srcs


def _assign_slots(nodes, wvec, T):
    """Sort nodes by per-window count vector (lexsort, first window most
    significant last => np.lexsort(wvec.T)), pads first."""
    order = np.lexsort(wvec.T)
    slot_node = np.full(T * P, -1, np.int64)
    slot_node[T * P - len(nodes):] = nodes[order]
    return slot_node


def _dprof_w(slot_node, node_wvec_lookup, T, NW):
    """Per-tile per-window max counts. node_wvec_lookup: [n_universe, NW]."""
    vec = np.zeros((T * P, NW), np.int64)
    v = slot_node >= 0
    vec[v] = node_wvec_lookup[slot_node[v]]
    return vec.reshape(T, P, NW).max(axis=1)


def _fill_idx(slot_node, D, colbase, Ctot, degi, s_sorted, indptr,
              src_key, win):
    """Build the flat int16 gather index list [Ctot*128] (0 = window zero
    row), then wrap for dma_gather: [128, Ctot*8]."""
    flat = np.zeros(Ctot * P, np.int16)
    k = np.nonzero(slot_node >= 0)[0]
    nodes = slot_node[k]
    rep, jj, srcs = _edge_expand(nodes, degi, s_sorted, indptr)
    keys = src_key[srcs]
    w_e = keys // win
    loc = keys % win + 1
    slot = k[rep]
    tt = slot // P
    pp = slot % P
    # rank within (edge's slot, window): edges of one node are contiguous in
    # rep order; stable sort by window within each node run.
    o = np.lexsort((jj, w_e, rep))
    so_rep, so_w = rep[o], w_e[o]
    grp_change = np.ones(len(o), bool)
    grp_change[1:] = (so_rep[1:] != so_rep[:-1]) | (so_w[1:] != so_w[:-1])
    gid = np.cumsum(grp_change) - 1
    starts = np.nonzero(grp_change)[0]
    rank_sorted = np.arange(len(o)) - starts[gid]
    rank = np.empty(len(o), np.int64)
    rank[o] = rank_sorted
    cb = np.asarray(colbase, np.int64)
    col = cb[tt, w_e] + rank
    flat[col * P + pp] = loc.astype(np.int16)
    wrapped = np.tile(flat.reshape(-1, 16).T, (8, 1))
    return flat, wrapped


def _preprocess(x, edge_index, batch, num_graphs, W0, b0, W1, b1, W2, b2):
    x = np.asarray(x, np.float32)
    N, IN = x.shape
    HID = W0.shape[1]
    G = int(num_graphs)
    SH = N // NCORES
    src = np.asarray(edge_index[0], np.int64)
    dst = np.asarray(edge_index[1], np.int64)
    batch = np.asarray(batch, np.int64)

    degi = np.bincount(dst, minlength=N) + 1          # + self-loop
    dis = (1.0 / np.sqrt(degi.astype(np.float64))).astype(np.float32)
    invd = np.sqrt(degi.astype(np.float64)).astype(np.float32)

    order = np.argsort(dst, kind="stable")
    s_sorted = src[order]
    indptr = np.searchsorted(dst, np.arange(N + 1), sorter=order)

    xhat = (x * dis[:, None]).astype(np.float32)

    # dense pooling matrix Mhat = diag(1/cnt) @ S_pool @ A_norm  [G, N]
    cnt = np.bincount(batch, minlength=G).astype(np.float64)
    cntc = np.maximum(cnt, 1.0)
    bd = batch[dst]
    w_ = dis[dst].astype(np.float64) * dis[src] / cntc[bd]
    M = np.bincount(bd * N + src, weights=w_, minlength=G * N)
    w2_ = dis.astype(np.float64) ** 2 / cntc[batch]
    M += np.bincount(batch * N + np.arange(N), weights=w2_, minlength=G * N)
    Mhat = M.reshape(G, N).astype(np.float32)

    halos, owns = [], []
    for c in range(NCORES):
        own = np.arange(c * SH, (c + 1) * SH)
        halo = np.unique(np.concatenate(
            [s_sorted[indptr[c * SH]:indptr[(c + 1) * SH]], own]))
        owns.append(own)
        halos.append(halo)

    # Layer 0: degree-sorted halo slots so all cores share one per-tile
    # column profile D0 with minimal padding.
    T0 = max(-(-len(h) // P) for h in halos)
    T0 = -(-T0 // 4) * 4
    assert T0 * P <= 2 * WIN1, "layer-1 source exceeds two windows"
    NW1 = -(-(T0 * P) // WIN1)
    T1 = -(-(-(-SH // P)) // 4) * 4

    slot0 = []
    for c in range(NCORES):
        h = halos[c]
        o = np.argsort(-degi[h], kind="stable")
        sn = np.full(T0 * P, -1, np.int64)
        sn[:len(h)] = h[o]
        slot0.append(sn)
    degmat = np.zeros((NCORES, T0 * P), np.int64)
    for c in range(NCORES):
        v = slot0[c] >= 0
        degmat[c][v] = degi[slot0[c][v]]
    D0 = degmat.reshape(NCORES, T0, P).max(axis=(0, 2))  # [T0]
    D0t = tuple(int(v) for v in D0)
    groups0, colbase0 = _schedule0(D0t)
    C0 = colbase0[T0]

    # layer-1 window vectors depend on each core's own slot0 (src slot ids)
    pos0s, wvec1s = [], []
    for c in range(NCORES):
        pos0 = np.full(N, -1, np.int64)
        v = slot0[c] >= 0
        pos0[slot0[c][v]] = np.nonzero(v)[0]
        pos0s.append(pos0)
        rep, _, srcs = _edge_expand(owns[c], degi, s_sorted, indptr)
        wv = np.zeros((SH, NW1), np.int64)
        np.add.at(wv, (rep, pos0[srcs] // WIN1), 1)
        wvec1s.append(wv)
    slot1 = [_assign_slots(owns[c], wvec1s[c], T1) for c in range(NCORES)]
    wvec1_full = []
    for c in range(NCORES):
        full = np.zeros((N, NW1), np.int64)
        full[owns[c]] = wvec1s[c]
        wvec1_full.append(full)
    D1 = np.max([_dprof_w(slot1[c], wvec1_full[c], T1, NW1)
                 for c in range(NCORES)], axis=0)
    D1t = tuple(tuple(int(v) for v in row) for row in D1)
    _, _, colbase1, C1 = _schedule(D1t, NW1)

    cb0 = np.asarray(colbase0[:T0], np.int64)
    cores = []
    for c in range(NCORES):
        s0, s1 = slot0[c], slot1[c]
        v0, v1 = s0 >= 0, s1 >= 0

        # layer-0 edge-expanded message table [P, C0*IN]
        X0 = np.zeros((C0, P, IN), np.float32)
        k = np.nonzero(v0)[0]
        nodes = s0[k]
        rep, jj, srcs = _edge_expand(nodes, degi, s_sorted, indptr)
        slot = k[rep]
        col = cb0[slot // P] + jj
        X0[col, slot % P] = xhat[srcs]
        x0 = np.clip(np.ascontiguousarray(
            X0.transpose(1, 0, 2).reshape(P, C0 * IN)), -240, 240).astype(F8)

        flat1, idx1 = _fill_idx(s1, D1t, colbase1, C1, degi, s_sorted, indptr,
                                pos0s[c], WIN1)

        def _scales(s, v, T, vec):
            d = np.where(v, vec[np.clip(s, 0, None)], 0).astype(np.float32)
            iv = np.where(v, invd[np.clip(s, 0, None)], 0).astype(np.float32)
            return d.reshape(T, P).T.copy(), iv.reshape(1, T * P)

        # layer-0 output is stored pre-scaled for the next gather:
        # h1_hat = dis*relu(dis*(u@W0)+b0) = relu(dis^2*(u@W0)+dis*b0)
        dis0, inv0 = _scales(s0, v0, T0, dis * dis)
        dis1, inv1 = _scales(s1, v1, T1, dis)
        mct = np.zeros((T1 * P, G), np.float32)
        mct[v1] = Mhat[:, s1[v1]].T
        cores.append(dict(x0=x0, idx1=idx1, dis0=dis0, dis1=dis1,
                          inv0=inv0.astype(BF16), inv1=inv1.astype(BF16),
                          mct=mct.astype(BF16), flat1=flat1))

    shared = dict(
        w0=np.ascontiguousarray(W0, np.float32).astype(BF16),
        w1=np.ascontiguousarray(W1, np.float32).reshape(2, P, HID).astype(BF16),
        b0r=np.ascontiguousarray(b0, np.float32).reshape(1, HID).astype(BF16),
        b1r=np.ascontiguousarray(b1, np.float32).reshape(1, HID).astype(BF16),
        ident=np.eye(P, dtype=np.float32).astype(BF16),
    )
    zero_bias = bool(np.all(np.asarray(b0) == 0) and np.all(np.asarray(b1) == 0))
    meta = dict(N=N, IN=IN, HID=HID, G=G, SH=SH, T0=T0, T1=T1,
                NW1=NW1, C0=C0, C1=C1, zero_bias=zero_bias,
                D0=D0t, D1=D1t)
    fin = dict(W2=np.asarray(W2, np.float32), b2=np.asarray(b2, np.float32))
    return meta, shared, cores, fin


# ---------------------------------------------------------------------------
# Pure-numpy emulation of the device program (validation / debugging)
# ---------------------------------------------------------------------------

def _emulate(meta, shared, cores, fin):
    T0, T1, HID, IN, G = (meta[k] for k in ("T0", "T1", "HID", "IN", "G"))
    NW1 = meta["NW1"]
    D0, D1 = meta["D0"], meta["D1"]
    _, colbase0 = _schedule0(D0)
    _, calls1, colbase1, C1 = _schedule(D1, NW1)
    w0 = shared["w0"].astype(np.float32)
    w1 = shared["w1"].astype(np.float32).reshape(2 * P, HID)
    b0 = shared["b0r"].astype(np.float32)[0]
    b1 = shared["b1r"].astype(np.float32)[0]

    def col_windows(calls, C):
        cw = np.zeros(C, np.int64)
        for w, n, off in calls:
            cw[off:off + n] = w
        return cw

    cw1 = col_windows(calls1, C1)

    Y = np.zeros((G, HID), np.float32)
    for cd in cores:
        tab = cd["x0"].astype(np.float32).reshape(P, -1, IN).transpose(1, 0, 2)
        u0 = np.zeros((T0 * P, IN), np.float32)
        for t in range(T0):
            for j in range(D0[t]):
                u0[t * P:(t + 1) * P] += tab[colbase0[t] + j]
        u0 = u0.astype(BF16).astype(np.float32)
        inv0 = cd["inv0"].astype(np.float32)[0]
        pre = u0 @ w0 + inv0[:, None] * b0[None, :]
        d0 = cd["dis0"].T.reshape(-1)
        h1 = np.maximum(d0[:, None] * pre, 0).astype(BF16).astype(np.float32)
        # place h1 into windowed layout
        h1w = np.zeros((NW1 * WSTR, HID), np.float32)
        for w in range(NW1):
            sl0, sl1 = w * WIN1, min((w + 1) * WIN1, T0 * P)
            h1w[w * WSTR + 1: w * WSTR + 1 + (sl1 - sl0)] = h1[sl0:sl1]

        rows1 = cw1.repeat(P) * WSTR + cd["flat1"]
        gat1 = h1w[rows1].reshape(C1, P, HID)
        u1 = np.zeros((T1 * P, HID), np.float32)
        for t in range(T1):
            for w in range(NW1):
                cb = colbase1[t][w]
                for j in range(D1[t][w]):
                    u1[t * P:(t + 1) * P] += gat1[cb + j]
        u1 = u1.astype(BF16).astype(np.float32)
        inv1 = cd["inv1"].astype(np.float32)[0]
        pre1 = u1 @ w1 + inv1[:, None] * b1[None, :]
        d1 = cd["dis1"].T.reshape(-1)
        h2 = np.maximum(d1[:, None] * pre1, 0).astype(BF16).astype(np.float32)
        Y += cd["mct"].astype(np.float32).T @ h2
    return Y @ fin["W2"] + fin["b2"]


# ---------------------------------------------------------------------------
# Bass device program
# ---------------------------------------------------------------------------

def _build(meta):
    import concourse.bass as bass
    import concourse.mybir as mybir
    import concourse.tile as tile
    from concourse import bacc, library_config
    from concourse.tile_rust import add_dep_helper

    F32, I16 = mybir.dt.float32, mybir.dt.int16
    BF = mybir.dt.bfloat16
    F8D = mybir.dt.float8e4
    RELU = mybir.ActivationFunctionType.Relu

    IN, HID, G = meta["IN"], meta["HID"], meta["G"]
    T0, T1 = meta["T0"], meta["T1"]
    NW1 = meta["NW1"]
    C0, C1 = meta["C0"], meta["C1"]
    D0, D1 = meta["D0"], meta["D1"]
    ZB = meta["zero_bias"]
    groups0, colbase0 = _schedule0(D0)
    groups1, calls1, colbase1, _ = _schedule(D1, NW1)
    c2c1 = _call_of_col(calls1)

    nc = bacc.Bacc("TRN2", target_bir_lowering=False, debug=False,
                   num_devices=NCORES)

    t_x0 = nc.dram_tensor("x0", [P, C0 * IN], F8D, kind="ExternalInput")
    t_idx1 = nc.dram_tensor("idx1", [P, C1 * 8], I16, kind="ExternalInput")
    t_dis0 = nc.dram_tensor("dis0", [P, T0], F32, kind="ExternalInput")
    t_dis1 = nc.dram_tensor("dis1", [P, T1], F32, kind="ExternalInput")
    t_inv0 = nc.dram_tensor("inv0", [1, T0 * P], BF, kind="ExternalInput")
    t_inv1 = nc.dram_tensor("inv1", [1, T1 * P], BF, kind="ExternalInput")
    t_w0 = nc.dram_tensor("w0", [IN, HID], BF, kind="ExternalInput")
    t_w1 = nc.dram_tensor("w1", [2, P, HID], BF, kind="ExternalInput")
    t_b0 = nc.dram_tensor("b0r", [1, HID], BF, kind="ExternalInput")
    t_b1 = nc.dram_tensor("b1r", [1, HID], BF, kind="ExternalInput")
    t_mct = nc.dram_tensor("mct", [T1 * P, G], BF, kind="ExternalInput")
    t_id = nc.dram_tensor("ident", [P, P], BF, kind="ExternalInput")
    t_out = nc.dram_tensor("outp", [G, HID], F32, kind="ExternalOutput")

    with tile.TileContext(nc) as tc:
        with (
            tc.tile_pool(name="const", bufs=1) as cpool,
            tc.tile_pool(name="ut", bufs=6) as upool,
            tc.tile_pool(name="stage", bufs=3) as spool,
            tc.tile_pool(name="aggps", bufs=4, space="PSUM") as apool,
            tc.tile_pool(name="preps", bufs=2, space="PSUM") as ppool,
            tc.tile_pool(name="outps", bufs=1, space="PSUM") as opool,
            tc.tile_pool(name="h2sbp", bufs=1) as h2pool,
            tc.tile_pool(name="dram", bufs=1, space="DRAM") as dpool,
        ):
            lib = nc.gpsimd.load_library(library_config.mlp)

            def cload(t, shape, dt):
                s = cpool.tile(shape, dt, tag=t.name)
                nc.sync.dma_start(s[:], t[:])
                return s

            ident = cload(t_id, [P, P], BF)
            w0 = cload(t_w0, [IN, HID], BF)
            w1 = cpool.tile([P, 2, HID], BF, tag="w1")
            nc.sync.dma_start(w1[:], t_w1[:].rearrange("j p h -> p j h"))
            if not ZB:
                b0r = cload(t_b0, [1, HID], BF)
                b1r = cload(t_b1, [1, HID], BF)
            dis0 = cload(t_dis0, [P, T0], F32)
            dis1 = cload(t_dis1, [P, T1], F32)

            h1h = dpool.tile([NW1 * WSTR, HID], BF)

            # h1h_writes[w]: writes a layer-1 gather from window w must wait on
            h1h_writes = [[] for _ in range(NW1)]
            zt = spool.tile([P, HID], BF, tag="zrow")
            nc.vector.memset(zt[:], 0.0)
            for w in range(NW1):
                h1h_writes[w].append(nc.sync.dma_start(
                    h1h[w * WSTR:w * WSTR + 1, :], zt[0:1, :]))

            def slot_row(s):
                return (s // WIN1) * WSTR + 1 + (s % WIN1)

            h2sb = h2pool.tile([P, T1 * HID], BF, tag="h2sb")

            # ---------------- Layer 0 ----------------
            with tc.tile_pool(name="x0p", bufs=3) as xpool:
                stage_t = None
                for (t0g, t1g) in groups0:
                    off = colbase0[t0g]
                    ncol = colbase0[t1g] - off
                    xt = xpool.tile([P, GC0 * IN], F8D, tag="x0")
                    nc.sync.dma_start(xt[:, :ncol * IN],
                                      t_x0[:, off * IN:(off + ncol) * IN])
                    for t in range(t0g, t1g):
                        nd = D0[t]
                        base = colbase0[t] - off
                        ups = apool.tile([P, P], mybir.dt.float32,
                                         tag="aggps", space="PSUM")
                        for j in range(nd):
                            nc.tensor.matmul(
                                ups[:],
                                lhsT=xt[:, (base + j) * IN:(base + j + 1) * IN],
                                rhs=ident[:], start=(j == 0),
                                stop=(j == nd - 1))
                        uT = upool.tile([P, P], BF, tag="ut")
                        if nd == 0:
                            nc.vector.memset(uT[:], 0.0)
                        else:
                            nc.vector.tensor_copy(uT[:], ups[:])
                        pre = ppool.tile([P, HID], mybir.dt.float32,
                                         tag="preps", space="PSUM")
                        if ZB:
                            nc.tensor.matmul(pre[:], lhsT=uT[:], rhs=w0[:],
                                             start=True, stop=True)
                        else:
                            sd = upool.tile([1, P], BF, tag="seed")
                            nc.sync.dma_start(sd[:],
                                              t_inv0[:, t * P:(t + 1) * P])
                            nc.tensor.matmul(pre[:], lhsT=sd[:], rhs=b0r[:],
                                             start=True, stop=False)
                            nc.tensor.matmul(pre[:], lhsT=uT[:], rhs=w0[:],
                                             start=False, stop=True)
                        sl = t % 4
                        if sl == 0:
                            stage_t = spool.tile([P, 4 * HID], BF,
                                                 tag="h1stage")
                        nc.scalar.activation(
                            stage_t[:, sl * HID:(sl + 1) * HID], pre[:],
                            RELU, bias=0.0, scale=dis0[:, t:t + 1])
                        if sl == 3:
                            s0r = (t - 3) * P
                            r0 = slot_row(s0r)
                            h1h_writes[s0r // WIN1].append(nc.sync.dma_start(
                                h1h[r0:r0 + 4 * P, :].rearrange(
                                    "(j p) h -> p j h", p=P),
                                stage_t[:].rearrange("p (j h) -> p j h", j=4)))

            # ---------------- Layer 1 ----------------
            with tc.tile_pool(name="idx1p", bufs=1) as ipool1, \
                 tc.tile_pool(name="g1", bufs=6) as gpool1:
                idx1 = ipool1.tile([P, C1 * 8], I16, tag="idx1")
                nc.sync.dma_start(idx1[:], t_idx1[:])

                def do_gathers(t_src_ap, idx_sb, calls, grp_calls, buf_pool,
                               ccap_elem, elem, deps_by_w):
                    out = {}
                    for ci in grp_calls:
                        w, ncols, off = calls[ci]
                        gt = buf_pool.tile([P, ccap_elem], BF, tag="g")
                        gi = nc.gpsimd.dma_gather(
                            gt[:, :ncols * elem].rearrange(
                                "p (j d) -> p j d", j=ncols),
                            t_src_ap(w),
                            idx_sb[:, off * 8:(off + ncols) * 8],
                            ncols * P, ncols * P, elem, single_packet=False)
                        add_dep_helper(gi.ins, lib.ins, True,
                                       "gather after lib")
                        for d in deps_by_w[w]:
                            add_dep_helper(gi.ins, d.ins, True,
                                           "gather after src")
                        out[ci] = gt
                    return out

                def grp_call_ids(calls, t0, t1, colbase, D, NW):
                    ids = set()
                    for tt in range(t0, t1):
                        for w in range(NW):
                            for j in range(D[tt][w]):
                                ids.add(c2c1[colbase[tt][w] + j][0])
                    return sorted(ids)

                for (t0g, t1g) in groups1:
                    ids = grp_call_ids(calls1, t0g, t1g, colbase1, D1, NW1)
                    bufs = do_gathers(
                        lambda w: h1h[w * WSTR:(w + 1) * WSTR, :],
                        idx1, calls1, ids, gpool1, CCAP * HID, HID,
                        h1h_writes)
                    for t in range(t0g, t1g):
                        nd = sum(D1[t])
                        u0ps = apool.tile([P, P], mybir.dt.float32,
                                          tag="aggps", space="PSUM")
                        u1ps = apool.tile([P, P], mybir.dt.float32,
                                          tag="aggps", space="PSUM")
                        k = 0
                        for w in range(NW1):
                            for j in range(D1[t][w]):
                                ci, lc = c2c1[colbase1[t][w] + j]
                                gt = bufs[ci]
                                nc.tensor.matmul(
                                    u0ps[:],
                                    lhsT=gt[:, lc * HID:lc * HID + P],
                                    rhs=ident[:], start=(k == 0),
                                    stop=(k == nd - 1))
                                nc.tensor.matmul(
                                    u1ps[:],
                                    lhsT=gt[:, lc * HID + P:(lc + 1) * HID],
                                    rhs=ident[:], start=(k == 0),
                                    stop=(k == nd - 1))
                                k += 1
                        uT0 = upool.tile([P, P], BF, tag="ut")
                        uT1 = upool.tile([P, P], BF, tag="ut")
                        if nd == 0:
                            nc.vector.memset(uT0[:], 0.0)
                            nc.vector.memset(uT1[:], 0.0)
                        else:
                            nc.vector.tensor_copy(uT0[:], u0ps[:])
                            nc.vector.tensor_copy(uT1[:], u1ps[:])
                        pre = ppool.tile([P, HID], mybir.dt.float32,
                                         tag="preps", space="PSUM")
                        if not ZB:
                            sd = upool.tile([1, P], BF, tag="seed")
                            nc.sync.dma_start(sd[:],
                                              t_inv1[:, t * P:(t + 1) * P])
                            nc.tensor.matmul(pre[:], lhsT=sd[:], rhs=b1r[:],
                                             start=True, stop=False)
                        nc.tensor.matmul(pre[:], lhsT=uT0[:], rhs=w1[:, 0, :],
                                         start=ZB, stop=False)
                        nc.tensor.matmul(pre[:], lhsT=uT1[:], rhs=w1[:, 1, :],
                                         start=False, stop=True)
                        nc.scalar.activation(
                            h2sb[:, t * HID:(t + 1) * HID], pre[:],
                            RELU, bias=0.0, scale=dis1[:, t:t + 1])

            # ---------------- Layer 2 + pool ----------------
            with tc.tile_pool(name="mc", bufs=3) as mpool:
                ops = opool.tile([G, HID], mybir.dt.float32, tag="outps",
                                 space="PSUM")
                for tb in range(0, T1, 4):
                    mt = mpool.tile([P, 4 * G], BF, tag="mc")
                    nc.sync.dma_start(
                        mt[:].rearrange("p (j g) -> p j g", j=4),
                        t_mct[tb * P:(tb + 4) * P, :].rearrange(
                            "(j p) g -> p j g", p=P))
                    for q in range(4):
                        t = tb + q
                        nc.tensor.matmul(
                            ops[:], lhsT=mt[:, q * G:(q + 1) * G],
                            rhs=h2sb[:, t * HID:(t + 1) * HID],
                            start=(t == 0), stop=(t == T1 - 1))
                osb = spool.tile([G, HID], mybir.dt.float32, tag="osb")
                nc.vector.tensor_copy(osb[:], ops[:])
                nc.sync.dma_start(t_out[:], osb[:])

    nc.compile()
    return nc


# ---------------------------------------------------------------------------
# Entry point
# ---------------------------------------------------------------------------

_cache = {}


def _get_nc(meta):
    key = hashlib.sha1(repr(sorted(meta.items())).encode()).hexdigest()
    if key not in _cache:
        _cache[key] = _build(meta)
    return _cache[key]


def _in_maps(shared, cores):
    maps = []
    for cd in cores:
        m = dict(shared)
        m.update({k: cd[k] for k in
                  ("x0", "idx1", "dis0", "dis1", "inv0", "inv1", "mct")})
        maps.append(m)
    return maps


def _run_device(meta, shared, cores):
    from concourse.bass_utils import run_bass_kernel_spmd
    nc = _get_nc(meta)
    res = run_bass_kernel_spmd(nc, _in_maps(shared, cores),
                               core_ids=list(range(NCORES)))
    return [r["outp"] for r in res.results]


def kernel(**inputs):
    meta, shared, cores, fin = _preprocess(**inputs)
    outs = _run_device(meta, shared, cores)
    Y = np.sum(np.stack(outs), axis=0, dtype=np.float32)
    out = Y @ fin["W2"] + fin["b2"]
    return out.astype(np.float32)


def profile_run(meta, shared, cores, trace_cores=None):
    """Profiled exec time in ns: NTFF trace when available, else the
    instruction-cost-model timeline simulation of the compiled program."""
    from concourse.bass_utils import run_bass_kernel_spmd
    nc = _get_nc(meta)
    try:
        res = run_bass_kernel_spmd(nc, _in_maps(shared, cores),
                                   core_ids=list(range(NCORES)), trace=True,
                                   trace_cores=trace_cores)
        if res.exec_time_ns is not None:
            print("profile:", res.instructions_and_trace[1]
                  if res.instructions_and_trace else None)
            return res.exec_time_ns
    except Exception as e:
        print(f"NTFF trace unavailable ({type(e).__name__}); "
              "using cost-model timeline")
    from concourse.timeline_sim import TimelineSim
    ts = TimelineSim(nc, trace=False)
    ts.simulate()
    return int(ts.time)


# revision 19
# speedup vs baseline: 2.0424x; 1.0614x over previous
"""GCN (3-layer GCNConv + global mean pool) on 8 Trainium2 NeuronCores.

Math: with S = adjacency+self-loops and D = diag(1/sqrt(deg)),
    conv(h) = relu(D S D h W + b)
and the diagonal scalings commute with the dense W, so each layer is an
UNWEIGHTED gather-sum of pre-scaled rows plus a dense matmul.  The final
conv + mean-pool collapse into a dense host-precomputed pooling matrix:
    out = (Mhat @ h2) @ W2 + b2,   Mhat = diag(1/cnt) S_pool A_norm.

Sharding: nodes dst-partitioned across 8 cores; layer 0 is recomputed on
each core's halo (src nodes of its incident edges) so cores never
communicate.  Layer 0 reads a host-prebuilt edge-expanded fp8 message
table laid out TRANSPOSED ([feature, slot]) in pairs, so one DoubleRow
matmul per message pair both aggregates and applies W0 (also fp8); the
per-node D^-1 scale is folded into the table and the dst-side D^-1/2 of
the pooling layer into Mhat, so every ReLU runs scale-free and batched
over 4 (resp. 2) tiles.  Layer 1 gathers h1 rows with the GPSIMD
dma_gather custom instruction (int16 indices -> sources split into
<=32767-row windows) and aggregates with identity transpose-matmuls.
Halo nodes are degree-sorted so the 8 cores share one program with
minimal padding; all index arithmetic happens on the host.
"""

import hashlib

import numpy as np
import ml_dtypes

P = 128
NCORES = 8
WSTR = 32768      # physical window stride (rows); row w*WSTR is all-zero
WIN1 = 32256      # usable slots per layer-1 source window (512-aligned)
GC0 = 64          # layer-0 table pair-columns per DMA chunk
GCOLS = 96        # layer-1 gather column budget per tile-group
CCAP = 32         # max columns per dma_gather call

BF16 = ml_dtypes.bfloat16
F8 = ml_dtypes.float8_e4m3    # TRN FP8_EXP4-compatible for |v| <= 240


def _f8(a):
    return np.clip(np.ascontiguousarray(a), -240, 240).astype(F8)


# ---------------------------------------------------------------------------
# Shared schedule derivation (host + builder + emulator all use this)
# ---------------------------------------------------------------------------

def _schedule(D, NW):
    """D: [T][NW] column counts.  Returns (groups, calls, colbase, Ctot).

    groups: list of (t0, t1) tile ranges with total columns <= GCOLS.
    calls: list of (w, ncols, col_off) in global column order; a call's
        columns are consecutive.  Global column order: per group, per
        window (ascending), per tile (ascending), per j.
    colbase: [T][NW] global column offset of (t, w)'s first column.
    """
    T = len(D)
    groups = []
    t = 0
    while t < T:
        tot = sum(D[t])
        t1 = t + 1
        while t1 < T and tot + sum(D[t1]) <= GCOLS:
            tot += sum(D[t1])
            t1 += 1
        groups.append((t, t1))
        t = t1
    colbase = [[0] * NW for _ in range(T)]
    calls = []
    off = 0
    for (t0, t1) in groups:
        for w in range(NW):
            cols = 0
            for tt in range(t0, t1):
                colbase[tt][w] = off + cols
                cols += D[tt][w]
            c0 = 0
            while c0 < cols:
                n = min(CCAP, cols - c0)
                calls.append((w, n, off + c0))
                c0 += n
            off += cols
    return groups, calls, colbase, off


def _schedule0(D0P):
    """Layer-0 chunking: greedy tile groups with <= GC0 pair-columns each.
    Returns (groups, colbase) with colbase[t] the global pair-col offset."""
    T = len(D0P)
    groups = []
    t = 0
    while t < T:
        tot = D0P[t]
        t1 = t + 1
        while t1 < T and tot + D0P[t1] <= GC0:
            tot += D0P[t1]
            t1 += 1
        groups.append((t, t1))
        t = t1
    colbase = [0] * (T + 1)
    for t in range(T):
        colbase[t + 1] = colbase[t] + D0P[t]
    return groups, colbase


def _call_of_col(calls):
    """Map global column -> (call_id, local_col)."""
    m = {}
    for ci, (w, n, off) in enumerate(calls):
        for j in range(n):
            m[off + j] = (ci, j)
    return m


# ---------------------------------------------------------------------------
# Host preprocessing
# ---------------------------------------------------------------------------

def _edge_expand(nodes, degi, s_sorted, indptr):
    """Expand in-edge lists (self-loop last) for `nodes`.
    Returns (slot_rep, jj, srcs): for each edge, owning node position in
    `nodes`, edge rank, and src global id."""
    dg = degi[nodes]
    tot = int(dg.sum())
    rep = np.repeat(np.arange(len(nodes)), dg)
    jj = np.arange(tot) - np.repeat(np.cumsum(dg) - dg, dg)
    g = nodes[rep]
    is_self = jj == (dg[rep] - 1)
    ei = np.minimum(indptr[g] + jj, len(s_sorted) - 1)
    srcs = np.where(is_self, g, s_sorted[ei])
    return rep, jj, srcs


def _assign_slots(nodes, wvec, T):
    """Sort nodes by per-window count vector (lexsort, first window most
    significant last => np.lexsort(wvec.T)), pads first."""
    order = np.lexsort(wvec.T)
    slot_node = np.full(T * P, -1, np.int64)
    slot_node[T * P - len(nodes):] = nodes[order]
    return slot_node


def _dprof_w(slot_node, node_wvec_lookup, T, NW):
    """Per-tile per-window max counts. node_wvec_lookup: [n_universe, NW]."""
    vec = np.zeros((T * P, NW), np.int64)
    v = slot_node >= 0
    vec[v] = node_wvec_lookup[slot_node[v]]
    return vec.reshape(T, P, NW).max(axis=1)


def _fill_idx(slot_node, D, colbase, Ctot, degi, s_sorted, indptr,
              src_key, win):
    """Build the flat int16 gather index list [Ctot*128] (0 = window zero
    row), then wrap for dma_gather: [128, Ctot*8]."""
    flat = np.zeros(Ctot * P, np.int16)
    k = np.nonzero(slot_node >= 0)[0]
    nodes = slot_node[k]
    rep, jj, srcs = _edge_expand(nodes, degi, s_sorted, indptr)
    keys = src_key[srcs]
    w_e = keys // win
    loc = keys % win + 1
    slot = k[rep]
    tt = slot // P
    pp = slot % P
    # rank within (edge's slot, window): edges of one node are contiguous in
    # rep order; stable sort by window within each node run.
    o = np.lexsort((jj, w_e, rep))
    so_rep, so_w = rep[o], w_e[o]
    grp_change = np.ones(len(o), bool)
    grp_change[1:] = (so_rep[1:] != so_rep[:-1]) | (so_w[1:] != so_w[:-1])
    gid = np.cumsum(grp_change) - 1
    starts = np.nonzero(grp_change)[0]
    rank_sorted = np.arange(len(o)) - starts[gid]
    rank = np.empty(len(o), np.int64)
    rank[o] = rank_sorted
    cb = np.asarray(colbase, np.int64)
    col = cb[tt, w_e] + rank
    flat[col * P + pp] = loc.astype(np.int16)
    wrapped = np.tile(flat.reshape(-1, 16).T, (8, 1))
    return flat, wrapped


def _preprocess(x, edge_index, batch, num_graphs, W0, b0, W1, b1, W2, b2):
    x = np.asarray(x, np.float32)
    N, IN = x.shape
    HID = W0.shape[1]
    G = int(num_graphs)
    SH = N // NCORES
    src = np.asarray(edge_index[0], np.int64)
    dst = np.asarray(edge_index[1], np.int64)
    batch = np.asarray(batch, np.int64)

    degi = np.bincount(dst, minlength=N) + 1          # + self-loop
    dis = (1.0 / np.sqrt(degi.astype(np.float64))).astype(np.float32)
    invd = np.sqrt(degi.astype(np.float64)).astype(np.float32)

    order = np.argsort(dst, kind="stable")
    s_sorted = src[order]
    indptr = np.searchsorted(dst, np.arange(N + 1), sorter=order)

    xhat = (x * dis[:, None]).astype(np.float32)
    dis2 = (dis * dis).astype(np.float32)

    # dense pooling matrix Mhat = diag(1/cnt) @ S_pool @ A_norm  [G, N]
    cnt = np.bincount(batch, minlength=G).astype(np.float64)
    cntc = np.maximum(cnt, 1.0)
    bd = batch[dst]
    w_ = dis[dst].astype(np.float64) * dis[src] / cntc[bd]
    M = np.bincount(bd * N + src, weights=w_, minlength=G * N)
    w2_ = dis.astype(np.float64) ** 2 / cntc[batch]
    M += np.bincount(batch * N + np.arange(N), weights=w2_, minlength=G * N)
    Mhat = M.reshape(G, N).astype(np.float32)

    halos, owns = [], []
    for c in range(NCORES):
        own = np.arange(c * SH, (c + 1) * SH)
        halo = np.unique(np.concatenate(
            [s_sorted[indptr[c * SH]:indptr[(c + 1) * SH]], own]))
        owns.append(own)
        halos.append(halo)

    # Layer 0: degree-sorted halo slots so all cores share one per-tile
    # column profile D0 with minimal padding.
    T0 = max(-(-len(h) // P) for h in halos)
    T0 = -(-T0 // 4) * 4
    assert T0 * P <= 2 * WIN1, "layer-1 source exceeds two windows"
    NW1 = -(-(T0 * P) // WIN1)
    T1 = -(-(-(-SH // P)) // 4) * 4

    slot0 = []
    for c in range(NCORES):
        h = halos[c]
        o = np.argsort(-degi[h], kind="stable")
        sn = np.full(T0 * P, -1, np.int64)
        sn[:len(h)] = h[o]
        slot0.append(sn)
    degmat = np.zeros((NCORES, T0 * P), np.int64)
    for c in range(NCORES):
        v = slot0[c] >= 0
        degmat[c][v] = degi[slot0[c][v]]
    D0 = degmat.reshape(NCORES, T0, P).max(axis=(0, 2))  # [T0]
    D0t = tuple(int(v) for v in D0)
    D0Pt = tuple(-(-int(v) // 2) for v in D0)            # pair columns
    groups0, colbase0 = _schedule0(D0Pt)
    C0P = colbase0[T0]

    # layer-1 window vectors depend on each core's own slot0 (src slot ids)
    pos0s, wvec1s = [], []
    for c in range(NCORES):
        pos0 = np.full(N, -1, np.int64)
        v = slot0[c] >= 0
        pos0[slot0[c][v]] = np.nonzero(v)[0]
        pos0s.append(pos0)
        rep, _, srcs = _edge_expand(owns[c], degi, s_sorted, indptr)
        wv = np.zeros((SH, NW1), np.int64)
        np.add.at(wv, (rep, pos0[srcs] // WIN1), 1)
        wvec1s.append(wv)
    slot1 = [_assign_slots(owns[c], wvec1s[c], T1) for c in range(NCORES)]
    wvec1_full = []
    for c in range(NCORES):
        full = np.zeros((N, NW1), np.int64)
        full[owns[c]] = wvec1s[c]
        wvec1_full.append(full)
    D1 = np.max([_dprof_w(slot1[c], wvec1_full[c], T1, NW1)
                 for c in range(NCORES)], axis=0)
    D1t = tuple(tuple(int(v) for v in row) for row in D1)
    _, _, colbase1, C1 = _schedule(D1t, NW1)

    cb0 = np.asarray(colbase0[:T0], np.int64)
    cores = []
    for c in range(NCORES):
        s0, s1 = slot0[c], slot1[c]
        v0, v1 = s0 >= 0, s1 >= 0

        # layer-0 fused message table: transposed pair layout, fp8.
        # entry [(cb0[t]+c)*2+i, slot, f] = dis2[dst] * xhat[src of msg 2c+i]
        X0 = np.zeros((C0P * 2, P, IN), np.float32)
        k = np.nonzero(v0)[0]
        nodes = s0[k]
        rep, jj, srcs = _edge_expand(nodes, degi, s_sorted, indptr)
        slot = k[rep]
        col2 = cb0[slot // P] * 2 + jj
        X0[col2, slot % P] = xhat[srcs] * dis2[nodes[rep]][:, None]
        # -> [f, col2, slot] -> [P, C0P*2*P]
        x0 = _f8(X0.transpose(2, 0, 1).reshape(P, C0P * 2 * P))

        flat1, idx1 = _fill_idx(s1, D1t, colbase1, C1, degi, s_sorted, indptr,
                                pos0s[c], WIN1)

        def _scales(s, v, T, vec):
            iv = np.where(v, vec[np.clip(s, 0, None)], 0).astype(np.float32)
            return iv.reshape(1, T * P)

        inv0 = _scales(s0, v0, T0, dis)     # bias seed for layer 0 (non-ZB)
        inv1 = _scales(s1, v1, T1, invd)    # bias seed for layer 1 (non-ZB)
        # dis1 (dst-side sqrt-deg scale of layer 2) folded into mct
        mct = np.zeros((T1 * P, G), np.float32)
        mct[v1] = (Mhat[:, s1[v1]] * dis[s1[v1]][None, :]).T
        cores.append(dict(x0=x0, idx1=idx1,
                          inv0=inv0.astype(BF16), inv1=inv1.astype(BF16),
                          mct=mct.astype(BF16), flat1=flat1))

    w0d = np.concatenate([np.asarray(W0, np.float32)] * 2, axis=1)  # [IN,2*HID]
    shared = dict(
        w0d=_f8(w0d),
        w1=np.ascontiguousarray(W1, np.float32).reshape(2, P, HID).astype(BF16),
        b0r=np.ascontiguousarray(b0, np.float32).reshape(1, HID).astype(BF16),
        b1r=np.ascontiguousarray(b1, np.float32).reshape(1, HID).astype(BF16),
        ident=np.eye(P, dtype=np.float32).astype(BF16),
    )
    zero_bias = bool(np.all(np.asarray(b0) == 0) and np.all(np.asarray(b1) == 0))
    meta = dict(N=N, IN=IN, HID=HID, G=G, SH=SH, T0=T0, T1=T1,
                NW1=NW1, C0P=C0P, C1=C1, zero_bias=zero_bias,
                D0P=D0Pt, D1=D1t)
    fin = dict(W2=np.asarray(W2, np.float32), b2=np.asarray(b2, np.float32))
    return meta, shared, cores, fin


# ---------------------------------------------------------------------------
# Pure-numpy emulation of the device program (validation / debugging)
# ---------------------------------------------------------------------------

def _emulate(meta, shared, cores, fin):
    T0, T1, HID, IN, G = (meta[k] for k in ("T0", "T1", "HID", "IN", "G"))
    NW1 = meta["NW1"]
    D0P, D1 = meta["D0P"], meta["D1"]
    _, colbase0 = _schedule0(D0P)
    C0P = colbase0[T0]
    _, calls1, colbase1, C1 = _schedule(D1, NW1)
    w0 = shared["w0d"].astype(np.float32)[:, :HID]
    w1 = shared["w1"].astype(np.float32).reshape(2 * P, HID)
    b0 = shared["b0r"].astype(np.float32)[0]
    b1 = shared["b1r"].astype(np.float32)[0]

    def col_windows(calls, C):
        cw = np.zeros(C, np.int64)
        for w, n, off in calls:
            cw[off:off + n] = w
        return cw

    cw1 = col_windows(calls1, C1)

    Y = np.zeros((G, HID), np.float32)
    for cd in cores:
        # table: [f, C0P*2, slot] -> [C0P*2, slot, f]
        tab = cd["x0"].astype(np.float32).reshape(P, C0P * 2, P)
        tab = tab.transpose(1, 2, 0)
        inv0 = cd["inv0"].astype(np.float32)[0]
        pre = inv0[:, None] * b0[None, :]
        pre = np.tile(pre.reshape(T0 * P, HID), (1, 1)).astype(np.float32)
        for t in range(T0):
            for c2 in range(D0P[t] * 2):
                pre[t * P:(t + 1) * P] += tab[colbase0[t] * 2 + c2] @ w0
        h1 = np.maximum(pre, 0).astype(BF16).astype(np.float32)
        # place h1 into windowed layout
        h1w = np.zeros((NW1 * WSTR, HID), np.float32)
        for w in range(NW1):
            sl0, sl1 = w * WIN1, min((w + 1) * WIN1, T0 * P)
            h1w[w * WSTR + 1: w * WSTR + 1 + (sl1 - sl0)] = h1[sl0:sl1]

        rows1 = cw1.repeat(P) * WSTR + cd["flat1"]
        gat1 = h1w[rows1].reshape(C1, P, HID)
        u1 = np.zeros((T1 * P, HID), np.float32)
        for t in range(T1):
            for w in range(NW1):
                cb = colbase1[t][w]
                for j in range(D1[t][w]):
                    u1[t * P:(t + 1) * P] += gat1[cb + j]
        u1 = u1.astype(BF16).astype(np.float32)
        inv1 = cd["inv1"].astype(np.float32)[0]
        pre1 = u1 @ w1 + inv1[:, None] * b1[None, :]
        h2 = np.maximum(pre1, 0).astype(BF16).astype(np.float32)
        Y += cd["mct"].astype(np.float32).T @ h2
    return Y @ fin["W2"] + fin["b2"]


# ---------------------------------------------------------------------------
# Bass device program
# ---------------------------------------------------------------------------

def _build(meta):
    import concourse.bass as bass
    import concourse.mybir as mybir
    import concourse.tile as tile
    from concourse import bacc, library_config
    from concourse.tile_rust import add_dep_helper

    F32, I16 = mybir.dt.float32, mybir.dt.int16
    BF = mybir.dt.bfloat16
    F8D = mybir.dt.float8e4
    RELU = mybir.ActivationFunctionType.Relu
    DR = mybir.MatmulPerfMode.DoubleRow

    IN, HID, G = meta["IN"], meta["HID"], meta["G"]
    T0, T1 = meta["T0"], meta["T1"]
    NW1 = meta["NW1"]
    C0P, C1 = meta["C0P"], meta["C1"]
    D0P, D1 = meta["D0P"], meta["D1"]
    ZB = meta["zero_bias"]
    groups0, colbase0 = _schedule0(D0P)
    groups1, calls1, colbase1, _ = _schedule(D1, NW1)
    c2c1 = _call_of_col(calls1)

    nc = bacc.Bacc("TRN2", target_bir_lowering=False, debug=False,
                   num_devices=NCORES)

    t_x0 = nc.dram_tensor("x0", [P, C0P * 2 * P], F8D, kind="ExternalInput")
    t_idx1 = nc.dram_tensor("idx1", [P, C1 * 8], I16, kind="ExternalInput")
    t_inv0 = nc.dram_tensor("inv0", [1, T0 * P], BF, kind="ExternalInput")
    t_inv1 = nc.dram_tensor("inv1", [1, T1 * P], BF, kind="ExternalInput")
    t_w0d = nc.dram_tensor("w0d", [IN, 2 * HID], F8D, kind="ExternalInput")
    t_w1 = nc.dram_tensor("w1", [2, P, HID], BF, kind="ExternalInput")
    t_b0 = nc.dram_tensor("b0r", [1, HID], BF, kind="ExternalInput")
    t_b1 = nc.dram_tensor("b1r", [1, HID], BF, kind="ExternalInput")
    t_mct = nc.dram_tensor("mct", [T1 * P, G], BF, kind="ExternalInput")
    t_id = nc.dram_tensor("ident", [P, P], BF, kind="ExternalInput")
    t_out = nc.dram_tensor("outp", [P, 2, G], F32, kind="ExternalOutput")

    with tile.TileContext(nc) as tc:
        with (
            tc.tile_pool(name="const", bufs=1) as cpool,
            tc.tile_pool(name="ut", bufs=6) as upool,
            tc.tile_pool(name="stage", bufs=3) as spool,
            tc.tile_pool(name="h2sbp", bufs=1) as h2pool,
            tc.tile_pool(name="dram", bufs=1, space="DRAM") as dpool,
        ):
            lib = nc.gpsimd.load_library(library_config.mlp)

            def cload(t, shape, dt):
                s = cpool.tile(shape, dt, tag=t.name)
                nc.sync.dma_start(s[:], t[:])
                return s

            ident = cload(t_id, [P, P], BF)
            w0d = cload(t_w0d, [IN, 2 * HID], F8D)
            w1 = cpool.tile([P, 2, HID], BF, tag="w1")
            nc.sync.dma_start(w1[:], t_w1[:].rearrange("j p h -> p j h"))
            if not ZB:
                b0r = cload(t_b0, [1, HID], BF)
                b1r = cload(t_b1, [1, HID], BF)

            h1h = dpool.tile([NW1 * WSTR, HID], BF)
            h2sb = h2pool.tile([P, T1 * HID], BF, tag="h2sb")

            # h1h_writes[w]: writes a layer-1 gather from window w must wait on
            h1h_writes = [[] for _ in range(NW1)]
            zt = spool.tile([P, HID], BF, tag="zrow")
            nc.vector.memset(zt[:], 0.0)
            for w in range(NW1):
                h1h_writes[w].append(nc.sync.dma_start(
                    h1h[w * WSTR:w * WSTR + 1, :], zt[0:1, :]))

            def slot_row(s):
                return (s // WIN1) * WSTR + 1 + (s % WIN1)

            # ---------------- Layer 0 ----------------
            with tc.tile_pool(name="x0p", bufs=3) as xpool, \
                 tc.tile_pool(name="pre0", bufs=2, space="PSUM") as ppool0:
                stage_t = None
                pre4 = None
                gi = 0
                xt = None
                off = 0
                for t in range(T0):
                    if gi < len(groups0) and t == groups0[gi][0]:
                        t0g, t1g = groups0[gi]
                        off = colbase0[t0g]
                        ncol = colbase0[t1g] - off
                        xt = xpool.tile([P, GC0 * 2 * P], F8D, tag="x0")
                        nc.sync.dma_start(
                            xt[:, :ncol * 2 * P],
                            t_x0[:, off * 2 * P:(off + ncol) * 2 * P])
                        gi += 1
                    q = t % 4
                    if q == 0:
                        pre4 = ppool0.tile([P, 4, 2 * HID], F32,
                                           tag="pre0", space="PSUM")
                    ncp = D0P[t]
                    base = colbase0[t] - off
                    if not ZB:
                        sd = upool.tile([1, P], BF, tag="seed")
                        nc.sync.dma_start(sd[:], t_inv0[:, t * P:(t + 1) * P])
                        nc.tensor.matmul(pre4[:, q, :HID], lhsT=sd[:],
                                         rhs=b0r[:], start=True, stop=False)
                    for c in range(ncp):
                        nc.tensor.matmul(
                            pre4[:, q, :HID],
                            lhsT=xt[:, (base + c) * 2 * P:(base + c + 1) * 2 * P
                                    ].rearrange("p (two m) -> p two m", two=2),
                            rhs=w0d[:].rearrange("p (two h) -> p two h", two=2),
                            start=(ZB and c == 0), stop=(c == ncp - 1),
                            perf_mode=DR)
                    if q == 3:
                        stage_t = spool.tile([P, 4 * HID], BF, tag="h1stage")
                        nc.scalar.activation(
                            stage_t[:].rearrange("p (j h) -> p j h", j=4),
                            pre4[:, :, :HID], RELU, bias=0.0, scale=1.0)
                        s0r = (t - 3) * P
                        r0 = slot_row(s0r)
                        h1h_writes[s0r // WIN1].append(nc.sync.dma_start(
                            h1h[r0:r0 + 4 * P, :].rearrange(
                                "(j p) h -> p j h", p=P),
                            stage_t[:].rearrange("p (j h) -> p j h", j=4)))

            # ---------------- Layer 1 ----------------
            with tc.tile_pool(name="idx1p", bufs=1) as ipool1, \
                 tc.tile_pool(name="g1", bufs=6) as gpool1, \
                 tc.tile_pool(name="aggps", bufs=4, space="PSUM") as apool, \
                 tc.tile_pool(name="pre1", bufs=2, space="PSUM") as ppool1:
                idx1 = ipool1.tile([P, C1 * 8], I16, tag="idx1")
                nc.sync.dma_start(idx1[:], t_idx1[:])

                def do_gathers(t_src_ap, idx_sb, calls, grp_calls, buf_pool,
                               ccap_elem, elem, deps_by_w):
                    out = {}
                    for ci in grp_calls:
                        w, ncols, off = calls[ci]
                        gt = buf_pool.tile([P, ccap_elem], BF, tag="g")
                        gi = nc.gpsimd.dma_gather(
                            gt[:, :ncols * elem].rearrange(
                                "p (j d) -> p j d", j=ncols),
                            t_src_ap(w),
                            idx_sb[:, off * 8:(off + ncols) * 8],
                            ncols * P, ncols * P, elem, single_packet=False)
                        add_dep_helper(gi.ins, lib.ins, True,
                                       "gather after lib")
                        for d in deps_by_w[w]:
                            add_dep_helper(gi.ins, d.ins, True,
                                           "gather after src")
                        out[ci] = gt
                    return out

                def grp_call_ids(calls, t0, t1, colbase, D, NW):
                    ids = set()
                    for tt in range(t0, t1):
                        for w in range(NW):
                            for j in range(D[tt][w]):
                                ids.add(c2c1[colbase[tt][w] + j][0])
                    return sorted(ids)

                pre2 = None
                for (t0g, t1g) in groups1:
                    ids = grp_call_ids(calls1, t0g, t1g, colbase1, D1, NW1)
                    bufs = do_gathers(
                        lambda w: h1h[w * WSTR:(w + 1) * WSTR, :],
                        idx1, calls1, ids, gpool1, CCAP * HID, HID,
                        h1h_writes)
                    for t in range(t0g, t1g):
                        nd = sum(D1[t])
                        u0ps = apool.tile([P, P], F32, tag="aggps",
                                          space="PSUM")
                        u1ps = apool.tile([P, P], F32, tag="aggps",
                                          space="PSUM")
                        k = 0
                        for w in range(NW1):
                            for j in range(D1[t][w]):
                                ci, lc = c2c1[colbase1[t][w] + j]
                                gt = bufs[ci]
                                nc.tensor.matmul(
                                    u0ps[:],
                                    lhsT=gt[:, lc * HID:lc * HID + P],
                                    rhs=ident[:], start=(k == 0),
                                    stop=(k == nd - 1))
                                nc.tensor.matmul(
                                    u1ps[:],
                                    lhsT=gt[:, lc * HID + P:(lc + 1) * HID],
                                    rhs=ident[:], start=(k == 0),
                                    stop=(k == nd - 1))
                                k += 1
                        uT0 = upool.tile([P, P], BF, tag="ut")
                        uT1 = upool.tile([P, P], BF, tag="ut")
                        if nd == 0:
                            nc.vector.memset(uT0[:], 0.0)
                            nc.vector.memset(uT1[:], 0.0)
                        else:
                            nc.vector.tensor_copy(uT0[:], u0ps[:])
                            nc.vector.tensor_copy(uT1[:], u1ps[:])
                        q = t % 2
                        if q == 0:
                            pre2 = ppool1.tile([P, 2, 2 * HID], F32,
                                               tag="pre1", space="PSUM")
                        if not ZB:
                            sd = upool.tile([1, P], BF, tag="seed")
                            nc.sync.dma_start(sd[:],
                                              t_inv1[:, t * P:(t + 1) * P])
                            nc.tensor.matmul(pre2[:, q, :HID], lhsT=sd[:],
                                             rhs=b1r[:], start=True,
                                             stop=False)
                        nc.tensor.matmul(pre2[:, q, :HID], lhsT=uT0[:],
                                         rhs=w1[:, 0, :],
                                         start=ZB, stop=False)
                        nc.tensor.matmul(pre2[:, q, :HID], lhsT=uT1[:],
                                         rhs=w1[:, 1, :],
                                         start=False, stop=True)
                        if q == 1:
                            nc.scalar.activation(
                                h2sb[:, (t - 1) * HID:(t + 1) * HID
                                     ].rearrange("p (j h) -> p j h", j=2),
                                pre2[:, :, :HID], RELU, bias=0.0, scale=1.0)

            # ---------------- Layer 2 + pool (transposed) ----------------
            with tc.tile_pool(name="mc", bufs=3) as mpool, \
                 tc.tile_pool(name="outps", bufs=1, space="PSUM") as opool:
                opsT0 = opool.tile([P, G], F32, tag="outps0", space="PSUM")
                opsT1 = opool.tile([P, G], F32, tag="outps1", space="PSUM")
                opsT = [opsT0, opsT1]
                for tb in range(0, T1, 4):
                    mt = mpool.tile([P, 4 * G], BF, tag="mc")
                    nc.sync.dma_start(
                        mt[:].rearrange("p (j g) -> p j g", j=4),
                        t_mct[tb * P:(tb + 4) * P, :].rearrange(
                            "(j p) g -> p j g", p=P))
                    for q in range(4):
                        t = tb + q
                        for fh in range(2):
                            nc.tensor.matmul(
                                opsT[fh][:],
                                lhsT=h2sb[:, t * HID + fh * P:
                                          t * HID + (fh + 1) * P],
                                rhs=mt[:, q * G:(q + 1) * G],
                                start=(t == 0), stop=(t == T1 - 1))
                osb = spool.tile([P, 2, G], F32, tag="osb")
                for fh in range(2):
                    nc.vector.tensor_copy(osb[:, fh, :], opsT[fh][:])
                nc.sync.dma_start(t_out[:], osb[:])

    nc.compile()
    return nc


# ---------------------------------------------------------------------------
# Entry point
# ---------------------------------------------------------------------------

_cache = {}


def _get_nc(meta):
    key = hashlib.sha1(repr(sorted(meta.items())).encode()).hexdigest()
    if key not in _cache:
        _cache[key] = _build(meta)
    return _cache[key]


def _in_maps(shared, cores):
    maps = []
    for cd in cores:
        m = dict(shared)
        m.update({k: cd[k] for k in
                  ("x0", "idx1", "inv0", "inv1", "mct")})
        maps.append(m)
    return maps


def _run_device(meta, shared, cores):
    from concourse.bass_utils import run_bass_kernel_spmd
    nc = _get_nc(meta)
    res = run_bass_kernel_spmd(nc, _in_maps(shared, cores),
                               core_ids=list(range(NCORES)))
    return [r["outp"] for r in res.results]


def kernel(**inputs):
    meta, shared, cores, fin = _preprocess(**inputs)
    outs = _run_device(meta, shared, cores)
    YT = np.sum(np.stack(outs), axis=0, dtype=np.float32)  # [P, 2, G]
    G = YT.shape[2]
    Y = YT.transpose(2, 1, 0).reshape(G, 2 * P)            # [G, HID]
    out = Y @ fin["W2"] + fin["b2"]
    return out.astype(np.float32)


def profile_run(meta, shared, cores, trace_cores=None):
    """Profiled exec time in ns: NTFF trace when available, else the
    instruction-cost-model timeline simulation of the compiled program."""
    from concourse.bass_utils import run_bass_kernel_spmd
    nc = _get_nc(meta)
    try:
        res = run_bass_kernel_spmd(nc, _in_maps(shared, cores),
                                   core_ids=list(range(NCORES)), trace=True,
                                   trace_cores=trace_cores)
        if res.exec_time_ns is not None:
            print("profile:", res.instructions_and_trace[1]
                  if res.instructions_and_trace else None)
            return res.exec_time_ns
    except Exception as e:
        print(f"NTFF trace unavailable ({type(e).__name__}); "
              "using cost-model timeline")
    from concourse.timeline_sim import TimelineSim
    ts = TimelineSim(nc, trace=False)
    ts.simulate()
    return int(ts.time)


# revision 28
# speedup vs baseline: 2.1829x; 1.0688x over previous
"""GCN (3-layer GCNConv + global mean pool) on 8 Trainium2 NeuronCores.

Math: with S = adjacency+self-loops and D = diag(1/sqrt(deg)),
    conv(h) = relu(D S D h W + b)
and the diagonal scalings commute with the dense W, so each layer is an
UNWEIGHTED gather-sum of pre-scaled rows plus a dense matmul.  The final
conv + mean-pool collapse into a dense host-precomputed pooling matrix:
    out = (Mhat @ h2) @ W2 + b2,   Mhat = diag(1/cnt) S_pool A_norm.

Sharding: nodes dst-partitioned across 8 cores; layer 0 is recomputed on
each core's halo (src nodes of its incident edges) so cores never
communicate.  Layer 0 reads a host-prebuilt edge-expanded fp8 message
table laid out TRANSPOSED ([feature, slot]) in pairs, so one DoubleRow
matmul per message pair both aggregates and applies W0 (also fp8); the
per-node D^-1 scale is folded into the table and the dst-side D^-1/2 of
the pooling layer into Mhat, so every ReLU runs scale-free and batched
over 4 (resp. 2) tiles.  Layer 1 gathers h1 rows with the GPSIMD
dma_gather custom instruction (int16 indices -> sources split into
<=32767-row windows) and aggregates with identity transpose-matmuls.
Halo nodes are degree-sorted so the 8 cores share one program with
minimal padding; all index arithmetic happens on the host.
"""

import hashlib

import numpy as np
import ml_dtypes

P = 128
NCORES = 8
WSTR = 32768      # physical window stride (rows); row w*WSTR is all-zero
WIN1 = 32256      # usable slots per layer-1 source window (512-aligned)
GC0 = 64          # layer-0 table pair-columns per DMA chunk
GCOLS = 64        # layer-1 gather column budget per tile-group
CCAP = 32         # max columns per dma_gather call

BF16 = ml_dtypes.bfloat16
F8 = ml_dtypes.float8_e4m3    # TRN FP8_EXP4-compatible for |v| <= 240


def _f8(a):
    return np.clip(np.ascontiguousarray(a), -240, 240).astype(F8)


# ---------------------------------------------------------------------------
# Shared schedule derivation (host + builder + emulator all use this)
# ---------------------------------------------------------------------------

def _schedule(D, NW):
    """D: [T][NW] column counts.  Returns (groups, calls, colbase, Ctot).

    groups: list of (t0, t1) tile ranges with total columns <= GCOLS.
    calls: list of (w, ncols, col_off) in global column order; a call's
        columns are consecutive.  Global column order: per group, per
        window (ascending), per tile (ascending), per j.
    colbase: [T][NW] global column offset of (t, w)'s first column.
    """
    T = len(D)
    groups = []
    t = 0
    while t < T:
        tot = sum(D[t])
        t1 = t + 1
        while t1 < T and tot + sum(D[t1]) <= GCOLS:
            tot += sum(D[t1])
            t1 += 1
        groups.append((t, t1))
        t = t1
    colbase = [[0] * NW for _ in range(T)]
    calls = []
    off = 0
    for (t0, t1) in groups:
        for w in range(NW):
            cols = 0
            for tt in range(t0, t1):
                colbase[tt][w] = off + cols
                cols += D[tt][w]
            c0 = 0
            while c0 < cols:
                n = min(CCAP, cols - c0)
                calls.append((w, n, off + c0))
                c0 += n
            off += cols
    return groups, calls, colbase, off


def _schedule0(D0P):
    """Layer-0 chunking: greedy tile groups with <= GC0 pair-columns each.
    Returns (groups, colbase) with colbase[t] the global pair-col offset."""
    T = len(D0P)
    groups = []
    t = 0
    while t < T:
        tot = D0P[t]
        t1 = t + 1
        while t1 < T and tot + D0P[t1] <= GC0:
            tot += D0P[t1]
            t1 += 1
        groups.append((t, t1))
        t = t1
    colbase = [0] * (T + 1)
    for t in range(T):
        colbase[t + 1] = colbase[t] + D0P[t]
    return groups, colbase


def _call_of_col(calls):
    """Map global column -> (call_id, local_col)."""
    m = {}
    for ci, (w, n, off) in enumerate(calls):
        for j in range(n):
            m[off + j] = (ci, j)
    return m


# ---------------------------------------------------------------------------
# Host preprocessing
# ---------------------------------------------------------------------------

def _edge_expand(nodes, degi, s_sorted, indptr):
    """Expand in-edge lists (self-loop last) for `nodes`.
    Returns (slot_rep, jj, srcs): for each edge, owning node position in
    `nodes`, edge rank, and src global id."""
    dg = degi[nodes]
    tot = int(dg.sum())
    rep = np.repeat(np.arange(len(nodes)), dg)
    jj = np.arange(tot) - np.repeat(np.cumsum(dg) - dg, dg)
    g = nodes[rep]
    is_self = jj == (dg[rep] - 1)
    ei = np.minimum(indptr[g] + jj, len(s_sorted) - 1)
    srcs = np.where(is_self, g, s_sorted[ei])
    return rep, jj, srcs


def _assign_slots(nodes, wvec, T):
    """Sort nodes by per-window count vector (lexsort, first window most
    significant last => np.lexsort(wvec.T)), pads first."""
    order = np.lexsort(wvec.T)
    slot_node = np.full(T * P, -1, np.int64)
    slot_node[T * P - len(nodes):] = nodes[order]
    return slot_node


def _dprof_w(slot_node, node_wvec_lookup, T, NW):
    """Per-tile per-window max counts. node_wvec_lookup: [n_universe, NW]."""
    vec = np.zeros((T * P, NW), np.int64)
    v = slot_node >= 0
    vec[v] = node_wvec_lookup[slot_node[v]]
    return vec.reshape(T, P, NW).max(axis=1)


def _fill_idx(slot_node, D, colbase, Ctot, degi, s_sorted, indptr,
              src_key, win):
    """Build the flat int16 gather index list [Ctot*128] (0 = window zero
    row), then wrap for dma_gather: [128, Ctot*8]."""
    flat = np.zeros(Ctot * P, np.int16)
    k = np.nonzero(slot_node >= 0)[0]
    nodes = slot_node[k]
    rep, jj, srcs = _edge_expand(nodes, degi, s_sorted, indptr)
    keys = src_key[srcs]
    w_e = keys // win
    loc = keys % win + 1
    slot = k[rep]
    tt = slot // P
    pp = slot % P
    # rank within (edge's slot, window): edges of one node are contiguous in
    # rep order; stable sort by window within each node run.
    o = np.lexsort((jj, w_e, rep))
    so_rep, so_w = rep[o], w_e[o]
    grp_change = np.ones(len(o), bool)
    grp_change[1:] = (so_rep[1:] != so_rep[:-1]) | (so_w[1:] != so_w[:-1])
    gid = np.cumsum(grp_change) - 1
    starts = np.nonzero(grp_change)[0]
    rank_sorted = np.arange(len(o)) - starts[gid]
    rank = np.empty(len(o), np.int64)
    rank[o] = rank_sorted
    cb = np.asarray(colbase, np.int64)
    col = cb[tt, w_e] + rank
    flat[col * P + pp] = loc.astype(np.int16)
    wrapped = np.tile(flat.reshape(-1, 16).T, (8, 1))
    return flat, wrapped


def _preprocess(x, edge_index, batch, num_graphs, W0, b0, W1, b1, W2, b2):
    x = np.asarray(x, np.float32)
    N, IN = x.shape
    HID = W0.shape[1]
    G = int(num_graphs)
    SH = N // NCORES
    src = np.asarray(edge_index[0], np.int64)
    dst = np.asarray(edge_index[1], np.int64)
    batch = np.asarray(batch, np.int64)

    degi = np.bincount(dst, minlength=N) + 1          # + self-loop
    dis = (1.0 / np.sqrt(degi.astype(np.float64))).astype(np.float32)
    invd = np.sqrt(degi.astype(np.float64)).astype(np.float32)

    order = np.argsort(dst, kind="stable")
    s_sorted = src[order]
    indptr = np.searchsorted(dst, np.arange(N + 1), sorter=order)

    xhat = (x * dis[:, None]).astype(np.float32)
    dis2 = (dis * dis).astype(np.float32)

    # dense pooling matrix Mhat = diag(1/cnt) @ S_pool @ A_norm  [G, N]
    cnt = np.bincount(batch, minlength=G).astype(np.float64)
    cntc = np.maximum(cnt, 1.0)
    bd = batch[dst]
    w_ = dis[dst].astype(np.float64) * dis[src] / cntc[bd]
    M = np.bincount(bd * N + src, weights=w_, minlength=G * N)
    w2_ = dis.astype(np.float64) ** 2 / cntc[batch]
    M += np.bincount(batch * N + np.arange(N), weights=w2_, minlength=G * N)
    Mhat = M.reshape(G, N).astype(np.float32)

    halos, owns = [], []
    for c in range(NCORES):
        own = np.arange(c * SH, (c + 1) * SH)
        halo = np.unique(np.concatenate(
            [s_sorted[indptr[c * SH]:indptr[(c + 1) * SH]], own]))
        owns.append(own)
        halos.append(halo)

    # Layer 0: degree-sorted halo slots so all cores share one per-tile
    # column profile D0 with minimal padding.
    T0 = max(-(-len(h) // P) for h in halos)
    T0 = -(-T0 // 4) * 4
    assert T0 * P <= 2 * WIN1, "layer-1 source exceeds two windows"
    NW1 = -(-(T0 * P) // WIN1)
    T1 = -(-(-(-SH // P)) // 4) * 4

    slot0 = []
    for c in range(NCORES):
        h = halos[c]
        o = np.argsort(-degi[h], kind="stable")
        sn = np.full(T0 * P, -1, np.int64)
        sn[:len(h)] = h[o]
        slot0.append(sn)
    degmat = np.zeros((NCORES, T0 * P), np.int64)
    for c in range(NCORES):
        v = slot0[c] >= 0
        degmat[c][v] = degi[slot0[c][v]]
    D0 = degmat.reshape(NCORES, T0, P).max(axis=(0, 2))  # [T0]
    D0t = tuple(int(v) for v in D0)
    D0Pt = tuple(-(-int(v) // 2) for v in D0)            # pair columns
    groups0, colbase0 = _schedule0(D0Pt)
    C0P = colbase0[T0]

    # layer-1 window vectors depend on each core's own slot0 (src slot ids)
    pos0s, wvec1s = [], []
    for c in range(NCORES):
        pos0 = np.full(N, -1, np.int64)
        v = slot0[c] >= 0
        pos0[slot0[c][v]] = np.nonzero(v)[0]
        pos0s.append(pos0)
        rep, _, srcs = _edge_expand(owns[c], degi, s_sorted, indptr)
        wv = np.zeros((SH, NW1), np.int64)
        np.add.at(wv, (rep, pos0[srcs] // WIN1), 1)
        wvec1s.append(wv)
    slot1 = [_assign_slots(owns[c], wvec1s[c], T1) for c in range(NCORES)]
    wvec1_full = []
    for c in range(NCORES):
        full = np.zeros((N, NW1), np.int64)
        full[owns[c]] = wvec1s[c]
        wvec1_full.append(full)
    D1 = np.max([_dprof_w(slot1[c], wvec1_full[c], T1, NW1)
                 for c in range(NCORES)], axis=0)
    D1t = tuple(tuple(int(v) for v in row) for row in D1)
    _, _, colbase1, C1 = _schedule(D1t, NW1)

    cb0 = np.asarray(colbase0[:T0], np.int64)
    cores = []
    for c in range(NCORES):
        s0, s1 = slot0[c], slot1[c]
        v0, v1 = s0 >= 0, s1 >= 0

        # layer-0 fused message table: transposed pair layout, fp8.
        # entry [(cb0[t]+c)*2+i, slot, f] = dis2[dst] * xhat[src of msg 2c+i]
        X0 = np.zeros((C0P * 2, P, IN), np.float32)
        k = np.nonzero(v0)[0]
        nodes = s0[k]
        rep, jj, srcs = _edge_expand(nodes, degi, s_sorted, indptr)
        slot = k[rep]
        col2 = cb0[slot // P] * 2 + jj
        X0[col2, slot % P] = xhat[srcs] * dis2[nodes[rep]][:, None]
        # -> [f, col2, slot] -> [P, C0P*2*P]
        x0 = _f8(X0.transpose(2, 0, 1).reshape(P, C0P * 2 * P))

        flat1, idx1 = _fill_idx(s1, D1t, colbase1, C1, degi, s_sorted, indptr,
                                pos0s[c], WIN1)

        def _scales(s, v, T, vec):
            iv = np.where(v, vec[np.clip(s, 0, None)], 0).astype(np.float32)
            return iv.reshape(1, T * P)

        inv0 = _scales(s0, v0, T0, dis)     # bias seed for layer 0 (non-ZB)
        inv1 = _scales(s1, v1, T1, invd)    # bias seed for layer 1 (non-ZB)
        # dis1 (dst-side sqrt-deg scale of layer 2) folded into mct
        mct = np.zeros((T1 * P, G), np.float32)
        mct[v1] = (Mhat[:, s1[v1]] * dis[s1[v1]][None, :]).T
        cores.append(dict(x0=x0, idx1=idx1,
                          inv0=inv0.astype(BF16), inv1=inv1.astype(BF16),
                          mct=mct.astype(BF16), flat1=flat1))

    w0d = np.concatenate([np.asarray(W0, np.float32)] * 2, axis=1)  # [IN,2*HID]
    shared = dict(
        w0d=_f8(w0d),
        w1=np.ascontiguousarray(W1, np.float32).reshape(2, P, HID).astype(BF16),
        b0r=np.ascontiguousarray(b0, np.float32).reshape(1, HID).astype(BF16),
        b1r=np.ascontiguousarray(b1, np.float32).reshape(1, HID).astype(BF16),
        ident=np.eye(P, dtype=np.float32).astype(BF16),
    )
    zero_bias = bool(np.all(np.asarray(b0) == 0) and np.all(np.asarray(b1) == 0))
    meta = dict(N=N, IN=IN, HID=HID, G=G, SH=SH, T0=T0, T1=T1,
                NW1=NW1, C0P=C0P, C1=C1, zero_bias=zero_bias,
                D0P=D0Pt, D1=D1t)
    fin = dict(W2=np.asarray(W2, np.float32), b2=np.asarray(b2, np.float32))
    return meta, shared, cores, fin


# ---------------------------------------------------------------------------
# Pure-numpy emulation of the device program (validation / debugging)
# ---------------------------------------------------------------------------

def _emulate(meta, shared, cores, fin):
    T0, T1, HID, IN, G = (meta[k] for k in ("T0", "T1", "HID", "IN", "G"))
    NW1 = meta["NW1"]
    D0P, D1 = meta["D0P"], meta["D1"]
    _, colbase0 = _schedule0(D0P)
    C0P = colbase0[T0]
    _, calls1, colbase1, C1 = _schedule(D1, NW1)
    w0 = shared["w0d"].astype(np.float32)[:, :HID]
    w1 = shared["w1"].astype(np.float32).reshape(2 * P, HID)
    b0 = shared["b0r"].astype(np.float32)[0]
    b1 = shared["b1r"].astype(np.float32)[0]

    def col_windows(calls, C):
        cw = np.zeros(C, np.int64)
        for w, n, off in calls:
            cw[off:off + n] = w
        return cw

    cw1 = col_windows(calls1, C1)

    Y = np.zeros((G, HID), np.float32)
    for cd in cores:
        # table: [f, C0P*2, slot] -> [C0P*2, slot, f]
        tab = cd["x0"].astype(np.float32).reshape(P, C0P * 2, P)
        tab = tab.transpose(1, 2, 0)
        inv0 = cd["inv0"].astype(np.float32)[0]
        pre = inv0[:, None] * b0[None, :]
        pre = np.tile(pre.reshape(T0 * P, HID), (1, 1)).astype(np.float32)
        for t in range(T0):
            for c2 in range(D0P[t] * 2):
                pre[t * P:(t + 1) * P] += tab[colbase0[t] * 2 + c2] @ w0
        h1 = np.maximum(pre, 0).astype(BF16).astype(np.float32)
        # place h1 into windowed layout
        h1w = np.zeros((NW1 * WSTR, HID), np.float32)
        for w in range(NW1):
            sl0, sl1 = w * WIN1, min((w + 1) * WIN1, T0 * P)
            h1w[w * WSTR + 1: w * WSTR + 1 + (sl1 - sl0)] = h1[sl0:sl1]

        rows1 = cw1.repeat(P) * WSTR + cd["flat1"]
        gat1 = h1w[rows1].reshape(C1, P, HID)
        u1 = np.zeros((T1 * P, HID), np.float32)
        for t in range(T1):
            for w in range(NW1):
                cb = colbase1[t][w]
                for j in range(D1[t][w]):
                    u1[t * P:(t + 1) * P] += gat1[cb + j]
        u1 = u1.astype(BF16).astype(np.float32)
        inv1 = cd["inv1"].astype(np.float32)[0]
        pre1 = u1 @ w1 + inv1[:, None] * b1[None, :]
        h2 = np.clip(np.maximum(pre1, 0), 0, 240).astype(F8).astype(np.float32)
        Y += cd["mct"].astype(np.float32).T @ h2
    return Y @ fin["W2"] + fin["b2"]


# ---------------------------------------------------------------------------
# Bass device program
# ---------------------------------------------------------------------------

def _build(meta):
    import concourse.bass as bass
    import concourse.mybir as mybir
    import concourse.tile as tile
    from concourse import bacc, library_config
    from concourse.tile_rust import add_dep_helper

    F32, I16 = mybir.dt.float32, mybir.dt.int16
    BF = mybir.dt.bfloat16
    F8D = mybir.dt.float8e4
    RELU = mybir.ActivationFunctionType.Relu
    DR = mybir.MatmulPerfMode.DoubleRow

    IN, HID, G = meta["IN"], meta["HID"], meta["G"]
    T0, T1 = meta["T0"], meta["T1"]
    NW1 = meta["NW1"]
    C0P, C1 = meta["C0P"], meta["C1"]
    D0P, D1 = meta["D0P"], meta["D1"]
    ZB = meta["zero_bias"]
    groups0, colbase0 = _schedule0(D0P)
    groups1, calls1, colbase1, _ = _schedule(D1, NW1)
    c2c1 = _call_of_col(calls1)

    nc = bacc.Bacc("TRN2", target_bir_lowering=False, debug=False,
                   num_devices=NCORES)

    t_x0 = nc.dram_tensor("x0", [P, C0P * 2 * P], F8D, kind="ExternalInput")
    t_idx1 = nc.dram_tensor("idx1", [P, C1 * 8], I16, kind="ExternalInput")
    t_inv0 = nc.dram_tensor("inv0", [1, T0 * P], BF, kind="ExternalInput")
    t_inv1 = nc.dram_tensor("inv1", [1, T1 * P], BF, kind="ExternalInput")
    t_w0d = nc.dram_tensor("w0d", [IN, 2 * HID], F8D, kind="ExternalInput")
    t_w1 = nc.dram_tensor("w1", [2, P, HID], BF, kind="ExternalInput")
    t_b0 = nc.dram_tensor("b0r", [1, HID], BF, kind="ExternalInput")
    t_b1 = nc.dram_tensor("b1r", [1, HID], BF, kind="ExternalInput")
    t_mct = nc.dram_tensor("mct", [T1 * P, G], BF, kind="ExternalInput")
    t_id = nc.dram_tensor("ident", [P, P], BF, kind="ExternalInput")
    t_out = nc.dram_tensor("outp", [P, 2, G], F32, kind="ExternalOutput")

    with tile.TileContext(nc) as tc:
        with (
            tc.tile_pool(name="const", bufs=1) as cpool,
            tc.tile_pool(name="ut", bufs=6) as upool,
            tc.tile_pool(name="stage", bufs=4) as spool,
            tc.tile_pool(name="h2sbp", bufs=1) as h2pool,
            tc.tile_pool(name="g1", bufs=5) as gpool1,
            tc.tile_pool(name="dram", bufs=1, space="DRAM") as dpool,
        ):
            lib = nc.gpsimd.load_library(library_config.mlp)

            def cload(t, shape, dt):
                s = cpool.tile(shape, dt, tag=t.name)
                nc.sync.dma_start(s[:], t[:])
                return s

            ident = cload(t_id, [P, P], BF)
            w0d = cload(t_w0d, [IN, 2 * HID], F8D)
            w1 = cpool.tile([P, 2, HID], BF, tag="w1")
            nc.sync.dma_start(w1[:], t_w1[:].rearrange("j p h -> p j h"))
            if not ZB:
                b0r = cload(t_b0, [1, HID], BF)
                b1r = cload(t_b1, [1, HID], BF)

            h1h = dpool.tile([NW1 * WSTR, HID], BF)
            h2sb = h2pool.tile([P, T1 * HID], F8D, tag="h2sb")

            # early loads on the Activation HWDGE queue: layer-1 indices +
            # pooling matrix
            idx1 = cpool.tile([P, C1 * 8], I16, tag="idx1")
            nc.scalar.dma_start(idx1[:], t_idx1[:])
            mct_all = cpool.tile([P, T1 * G], BF, tag="mct_all")
            nc.scalar.dma_start(
                mct_all[:].rearrange("p (t g) -> p t g", t=T1),
                t_mct[:].rearrange("(t p) g -> p t g", p=P))

            # h1h_writes[w]: writes a layer-1 gather from window w must wait on
            h1h_writes = [[] for _ in range(NW1)]
            zt = spool.tile([P, HID], BF, tag="zrow")
            nc.vector.memset(zt[:], 0.0)
            for w in range(NW1):
                h1h_writes[w].append(nc.scalar.dma_start(
                    h1h[w * WSTR:w * WSTR + 1, :], zt[0:1, :]))

            def slot_row(s):
                return (s // WIN1) * WSTR + 1 + (s % WIN1)

            # ---------------- Layer 0 ----------------
            with tc.tile_pool(name="x0p", bufs=3) as xpool, \
                 tc.tile_pool(name="pre0", bufs=2, space="PSUM") as ppool0:
                stage_t = None
                pre4 = None
                gi = 0
                xt = None
                off = 0
                for t in range(T0):
                    if gi < len(groups0) and t == groups0[gi][0]:
                        t0g, t1g = groups0[gi]
                        off = colbase0[t0g]
                        ncol = colbase0[t1g] - off
                        xt = xpool.tile([P, GC0 * 2 * P], F8D, tag="x0")
                        nc.sync.dma_start(
                            xt[:, :ncol * 2 * P],
                            t_x0[:, off * 2 * P:(off + ncol) * 2 * P])
                        gi += 1
                    q = t % 4
                    if q == 0:
                        pre4 = ppool0.tile([P, 4, 2 * HID], F32,
                                           tag="pre0", space="PSUM")
                    ncp = D0P[t]
                    base = colbase0[t] - off
                    if not ZB:
                        sd = upool.tile([1, P], BF, tag="seed")
                        nc.sync.dma_start(sd[:], t_inv0[:, t * P:(t + 1) * P])
                        nc.tensor.matmul(pre4[:, q, :HID], lhsT=sd[:],
                                         rhs=b0r[:], start=True, stop=False)
                    for c in range(ncp):
                        nc.tensor.matmul(
                            pre4[:, q, :HID],
                            lhsT=xt[:, (base + c) * 2 * P:(base + c + 1) * 2 * P
                                    ].rearrange("p (two m) -> p two m", two=2),
                            rhs=w0d[:].rearrange("p (two h) -> p two h", two=2),
                            start=(ZB and c == 0), stop=(c == ncp - 1),
                            perf_mode=DR)
                    if q == 3:
                        stage_t = spool.tile([P, 4 * HID], BF, tag="h1stage")
                        nc.scalar.activation(
                            stage_t[:].rearrange("p (j h) -> p j h", j=4),
                            pre4[:, :, :HID], RELU, bias=0.0, scale=1.0)
                        s0r = (t - 3) * P
                        r0 = slot_row(s0r)
                        h1h_writes[s0r // WIN1].append(nc.scalar.dma_start(
                            h1h[r0:r0 + 4 * P, :].rearrange(
                                "(j p) h -> p j h", p=P),
                            stage_t[:].rearrange("p (j h) -> p j h", j=4)))

            # ---------------- Layer 1 ----------------
            with tc.tile_pool(name="aggps", bufs=4, space="PSUM") as apool, \
                 tc.tile_pool(name="pre1", bufs=2, space="PSUM") as ppool1:

                def do_gathers(t_src_ap, idx_sb, calls, grp_calls, buf_pool,
                               ccap_elem, elem, deps_by_w):
                    out = {}
                    for ci in grp_calls:
                        w, ncols, off = calls[ci]
                        gt = buf_pool.tile([P, ccap_elem], BF, tag="g")
                        gi = nc.gpsimd.dma_gather(
                            gt[:, :ncols * elem].rearrange(
                                "p (j d) -> p j d", j=ncols),
                            t_src_ap(w),
                            idx_sb[:, off * 8:(off + ncols) * 8],
                            ncols * P, ncols * P, elem, single_packet=False)
                        add_dep_helper(gi.ins, lib.ins, True,
                                       "gather after lib")
                        for d in deps_by_w[w]:
                            add_dep_helper(gi.ins, d.ins, True,
                                           "gather after src")
                        out[ci] = gt
                    return out

                def grp_call_ids(calls, t0, t1, colbase, D, NW):
                    ids = set()
                    for tt in range(t0, t1):
                        for w in range(NW):
                            for j in range(D[tt][w]):
                                ids.add(c2c1[colbase[tt][w] + j][0])
                    return sorted(ids)

                pre2 = None
                for (t0g, t1g) in groups1:
                    ids = grp_call_ids(calls1, t0g, t1g, colbase1, D1, NW1)
                    bufs = do_gathers(
                        lambda w: h1h[w * WSTR:(w + 1) * WSTR, :],
                        idx1, calls1, ids, gpool1, CCAP * HID, HID,
                        h1h_writes)
                    for t in range(t0g, t1g):
                        nd = sum(D1[t])
                        u0ps = apool.tile([P, P], F32, tag="aggps",
                                          space="PSUM")
                        u1ps = apool.tile([P, P], F32, tag="aggps",
                                          space="PSUM")
                        k = 0
                        for w in range(NW1):
                            for j in range(D1[t][w]):
                                ci, lc = c2c1[colbase1[t][w] + j]
                                gt = bufs[ci]
                                nc.tensor.matmul(
                                    u0ps[:],
                                    lhsT=gt[:, lc * HID:lc * HID + P],
                                    rhs=ident[:], start=(k == 0),
                                    stop=(k == nd - 1))
                                nc.tensor.matmul(
                                    u1ps[:],
                                    lhsT=gt[:, lc * HID + P:(lc + 1) * HID],
                                    rhs=ident[:], start=(k == 0),
                                    stop=(k == nd - 1))
                                k += 1
                        uT0 = upool.tile([P, P], BF, tag="ut")
                        uT1 = upool.tile([P, P], BF, tag="ut")
                        if nd == 0:
                            nc.vector.memset(uT0[:], 0.0)
                            nc.vector.memset(uT1[:], 0.0)
                        else:
                            nc.vector.tensor_copy(uT0[:], u0ps[:])
                            nc.vector.tensor_copy(uT1[:], u1ps[:])
                        q = t % 2
                        if q == 0:
                            pre2 = ppool1.tile([P, 2, 2 * HID], F32,
                                               tag="pre1", space="PSUM")
                        if not ZB:
                            sd = upool.tile([1, P], BF, tag="seed")
                            nc.sync.dma_start(sd[:],
                                              t_inv1[:, t * P:(t + 1) * P])
                            nc.tensor.matmul(pre2[:, q, :HID], lhsT=sd[:],
                                             rhs=b1r[:], start=True,
                                             stop=False)
                        nc.tensor.matmul(pre2[:, q, :HID], lhsT=uT0[:],
                                         rhs=w1[:, 0, :],
                                         start=ZB, stop=False)
                        nc.tensor.matmul(pre2[:, q, :HID], lhsT=uT1[:],
                                         rhs=w1[:, 1, :],
                                         start=False, stop=True)
                        if q == 1:
                            nc.scalar.activation(
                                h2sb[:, (t - 1) * HID:(t + 1) * HID
                                     ].rearrange("p (j h) -> p j h", j=2),
                                pre2[:, :, :HID], RELU, bias=0.0, scale=1.0)

            # ---------------- Layer 2 + pool (transposed) ----------------
            with tc.tile_pool(name="outps", bufs=1, space="PSUM") as opool:
                opsT0 = opool.tile([P, G], F32, tag="outps0", space="PSUM")
                opsT1 = opool.tile([P, G], F32, tag="outps1", space="PSUM")
                opsT = [opsT0, opsT1]
                for t in range(T1):
                    for fh in range(2):
                        nc.tensor.matmul(
                            opsT[fh][:],
                            lhsT=h2sb[:, t * HID + fh * P:
                                      t * HID + (fh + 1) * P],
                            rhs=mct_all[:, t * G:(t + 1) * G],
                            start=(t == 0), stop=(t == T1 - 1))
                osb = spool.tile([P, 2, G], F32, tag="osb")
                for fh in range(2):
                    nc.vector.tensor_copy(osb[:, fh, :], opsT[fh][:])
                nc.sync.dma_start(t_out[:], osb[:])

    nc.compile()
    return nc


# ---------------------------------------------------------------------------
# Entry point
# ---------------------------------------------------------------------------

_cache = {}


def _get_nc(meta):
    key = hashlib.sha1(repr(sorted(meta.items())).encode()).hexdigest()
    if key not in _cache:
        _cache[key] = _build(meta)
    return _cache[key]


def _in_maps(shared, cores):
    maps = []
    for cd in cores:
        m = dict(shared)
        m.update({k: cd[k] for k in
                  ("x0", "idx1", "inv0", "inv1", "mct")})
        maps.append(m)
    return maps


def _run_device(meta, shared, cores):
    from concourse.bass_utils import run_bass_kernel_spmd
    nc = _get_nc(meta)
    res = run_bass_kernel_spmd(nc, _in_maps(shared, cores),
                               core_ids=list(range(NCORES)))
    return [r["outp"] for r in res.results]


def kernel(**inputs):
    meta, shared, cores, fin = _preprocess(**inputs)
    outs = _run_device(meta, shared, cores)
    YT = np.sum(np.stack(outs), axis=0, dtype=np.float32)  # [P, 2, G]
    G = YT.shape[2]
    Y = YT.transpose(2, 1, 0).reshape(G, 2 * P)            # [G, HID]
    out = Y @ fin["W2"] + fin["b2"]
    return out.astype(np.float32)


def profile_run(meta, shared, cores, trace_cores=None):
    """Profiled exec time in ns: NTFF trace when available, else the
    instruction-cost-model timeline simulation of the compiled program."""
    from concourse.bass_utils import run_bass_kernel_spmd
    nc = _get_nc(meta)
    try:
        res = run_bass_kernel_spmd(nc, _in_maps(shared, cores),
                                   core_ids=list(range(NCORES)), trace=True,
                                   trace_cores=trace_cores)
        if res.exec_time_ns is not None:
            print("profile:", res.instructions_and_trace[1]
                  if res.instructions_and_trace else None)
            return res.exec_time_ns
    except Exception as e:
        print(f"NTFF trace unavailable ({type(e).__name__}); "
              "using cost-model timeline")
    from concourse.timeline_sim import TimelineSim
    ts = TimelineSim(nc, trace=False)
    ts.simulate()
    return int(ts.time)


# revision 55
# speedup vs baseline: 2.1936x; 1.0049x over previous
"""GCN (3-layer GCNConv + global mean pool) on 8 Trainium2 NeuronCores.

Math: with S = adjacency+self-loops and D = diag(1/sqrt(deg)),
    conv(h) = relu(D S D h W + b)
and the diagonal scalings commute with the dense W, so each layer is an
UNWEIGHTED gather-sum of pre-scaled rows plus a dense matmul.  The final
conv + mean-pool collapse into a dense host-precomputed pooling matrix:
    out = (Mhat @ h2) @ W2 + b2,   Mhat = diag(1/cnt) S_pool A_norm.

Sharding: nodes dst-partitioned across 8 cores; layer 0 is recomputed on
each core's halo (src nodes of its incident edges) so cores never
communicate.  Layer 0 reads a host-prebuilt edge-expanded fp8 message
table laid out TRANSPOSED ([feature, slot]) in pairs, so one DoubleRow
matmul per message pair both aggregates and applies W0 (also fp8); the
per-node D^-1 scale is folded into the table and the dst-side D^-1/2 of
the pooling layer into Mhat, so every ReLU runs scale-free and batched
over 4 (resp. 2) tiles.  Layer 1 gathers h1 rows with the GPSIMD
dma_gather custom instruction (int16 indices -> sources split into
<=32767-row windows) and aggregates with identity transpose-matmuls.
Halo nodes are degree-sorted so the 8 cores share one program with
minimal padding; all index arithmetic happens on the host.
"""

import hashlib

import numpy as np
import ml_dtypes

P = 128
NCORES = 8
WSTR = 32768      # physical window stride (rows); row w*WSTR is all-zero
WIN1 = 32256      # usable slots per layer-1 source window (512-aligned)
GC0 = 64          # layer-0 table pair-columns per DMA chunk
GCOLS = 64        # layer-1 gather column budget per tile-group
CCAP = 32         # max columns per dma_gather call

BF16 = ml_dtypes.bfloat16
F8 = ml_dtypes.float8_e4m3    # TRN FP8_EXP4-compatible for |v| <= 240

# power-of-2 pre-scales keeping fp8 operands out of the subnormal range;
# compensated exactly by the fp32 activation scales and the pooling matrix
S_T, S_W0, S_H2 = 32.0, 32.0, 16.0
ACT0_SCALE = 1.0 / (S_T * S_W0)
ACT1_SCALE = S_H2


def _f8(a):
    return np.clip(np.ascontiguousarray(a), -240, 240).astype(F8)


# ---------------------------------------------------------------------------
# Shared schedule derivation (host + builder + emulator all use this)
# ---------------------------------------------------------------------------

def _schedule(D, NW):
    """D: [T][NW] column counts.  Returns (groups, calls, colbase, Ctot).

    groups: list of (t0, t1) tile ranges with total columns <= GCOLS.
    calls: list of (w, ncols, col_off) in global column order; a call's
        columns are consecutive.  Global column order: per group, per
        window (ascending), per tile (ascending), per j.
    colbase: [T][NW] global column offset of (t, w)'s first column.
    """
    T = len(D)
    groups = []
    t = 0
    while t < T:
        tot = sum(D[t])
        t1 = t + 1
        while t1 < T and tot + sum(D[t1]) <= GCOLS:
            tot += sum(D[t1])
            t1 += 1
        groups.append((t, t1))
        t = t1
    colbase = [[0] * NW for _ in range(T)]
    calls = []
    off = 0
    for (t0, t1) in groups:
        for w in range(NW):
            cols = 0
            for tt in range(t0, t1):
                colbase[tt][w] = off + cols
                cols += D[tt][w]
            c0 = 0
            while c0 < cols:
                n = min(CCAP, cols - c0)
                calls.append((w, n, off + c0))
                c0 += n
            off += cols
    return groups, calls, colbase, off


def _schedule0(D0P):
    """Layer-0 chunking: greedy tile groups with <= GC0 pair-columns each.
    Returns (groups, colbase) with colbase[t] the global pair-col offset."""
    T = len(D0P)
    groups = []
    t = 0
    while t < T:
        tot = D0P[t]
        t1 = t + 1
        while t1 < T and tot + D0P[t1] <= GC0:
            tot += D0P[t1]
            t1 += 1
        groups.append((t, t1))
        t = t1
    colbase = [0] * (T + 1)
    for t in range(T):
        colbase[t + 1] = colbase[t] + D0P[t]
    return groups, colbase


def _call_of_col(calls):
    """Map global column -> (call_id, local_col)."""
    m = {}
    for ci, (w, n, off) in enumerate(calls):
        for j in range(n):
            m[off + j] = (ci, j)
    return m


# ---------------------------------------------------------------------------
# Host preprocessing
# ---------------------------------------------------------------------------

def _edge_expand(nodes, degi, s_sorted, indptr):
    """Expand in-edge lists (self-loop last) for `nodes`.
    Returns (slot_rep, jj, srcs): for each edge, owning node position in
    `nodes`, edge rank, and src global id."""
    dg = degi[nodes]
    tot = int(dg.sum())
    rep = np.repeat(np.arange(len(nodes)), dg)
    jj = np.arange(tot) - np.repeat(np.cumsum(dg) - dg, dg)
    g = nodes[rep]
    is_self = jj == (dg[rep] - 1)
    ei = np.minimum(indptr[g] + jj, len(s_sorted) - 1)
    srcs = np.where(is_self, g, s_sorted[ei])
    return rep, jj, srcs


def _assign_slots(nodes, wvec, T):
    """Sort nodes by per-window count vector (lexsort, first window most
    significant last => np.lexsort(wvec.T)), pads first."""
    order = np.lexsort(wvec.T)
    slot_node = np.full(T * P, -1, np.int64)
    slot_node[T * P - len(nodes):] = nodes[order]
    return slot_node


def _dprof_w(slot_node, node_wvec_lookup, T, NW):
    """Per-tile per-window max counts. node_wvec_lookup: [n_universe, NW]."""
    vec = np.zeros((T * P, NW), np.int64)
    v = slot_node >= 0
    vec[v] = node_wvec_lookup[slot_node[v]]
    return vec.reshape(T, P, NW).max(axis=1)


def _fill_idx(slot_node, D, colbase, Ctot, degi, s_sorted, indptr,
              src_key, win):
    """Build the flat int16 gather index list [Ctot*128] (0 = window zero
    row), then wrap for dma_gather: [128, Ctot*8]."""
    flat = np.zeros(Ctot * P, np.int16)
    k = np.nonzero(slot_node >= 0)[0]
    nodes = slot_node[k]
    rep, jj, srcs = _edge_expand(nodes, degi, s_sorted, indptr)
    keys = src_key[srcs]
    w_e = keys // win
    loc = keys % win + 1
    slot = k[rep]
    tt = slot // P
    pp = slot % P
    # rank within (edge's slot, window): edges of one node are contiguous in
    # rep order; stable sort by window within each node run.
    o = np.lexsort((jj, w_e, rep))
    so_rep, so_w = rep[o], w_e[o]
    grp_change = np.ones(len(o), bool)
    grp_change[1:] = (so_rep[1:] != so_rep[:-1]) | (so_w[1:] != so_w[:-1])
    gid = np.cumsum(grp_change) - 1
    starts = np.nonzero(grp_change)[0]
    rank_sorted = np.arange(len(o)) - starts[gid]
    rank = np.empty(len(o), np.int64)
    rank[o] = rank_sorted
    cb = np.asarray(colbase, np.int64)
    col = cb[tt, w_e] + rank
    flat[col * P + pp] = loc.astype(np.int16)
    wrapped = np.tile(flat.reshape(-1, 16).T, (8, 1))
    return flat, wrapped


def _preprocess(x, edge_index, batch, num_graphs, W0, b0, W1, b1, W2, b2):
    x = np.asarray(x, np.float32)
    N, IN = x.shape
    HID = W0.shape[1]
    G = int(num_graphs)
    SH = N // NCORES
    src = np.asarray(edge_index[0], np.int64)
    dst = np.asarray(edge_index[1], np.int64)
    batch = np.asarray(batch, np.int64)

    degi = np.bincount(dst, minlength=N) + 1          # + self-loop
    dis = (1.0 / np.sqrt(degi.astype(np.float64))).astype(np.float32)
    invd = np.sqrt(degi.astype(np.float64)).astype(np.float32)

    order = np.argsort(dst, kind="stable")
    s_sorted = src[order]
    indptr = np.searchsorted(dst, np.arange(N + 1), sorter=order)

    xhat = (x * dis[:, None]).astype(np.float32)
    dis2 = (dis * dis).astype(np.float32)

    # dense pooling matrix Mhat = diag(1/cnt) @ S_pool @ A_norm  [G, N]
    cnt = np.bincount(batch, minlength=G).astype(np.float64)
    cntc = np.maximum(cnt, 1.0)
    bd = batch[dst]
    w_ = dis[dst].astype(np.float64) * dis[src] / cntc[bd]
    M = np.bincount(bd * N + src, weights=w_, minlength=G * N)
    w2_ = dis.astype(np.float64) ** 2 / cntc[batch]
    M += np.bincount(batch * N + np.arange(N), weights=w2_, minlength=G * N)
    Mhat = M.reshape(G, N).astype(np.float32)

    halos, owns = [], []
    for c in range(NCORES):
        own = np.arange(c * SH, (c + 1) * SH)
        halo = np.unique(np.concatenate(
            [s_sorted[indptr[c * SH]:indptr[(c + 1) * SH]], own]))
        owns.append(own)
        halos.append(halo)

    # Layer 0: degree-sorted halo slots so all cores share one per-tile
    # column profile D0 with minimal padding.
    T0 = max(-(-len(h) // P) for h in halos)
    T0 = -(-T0 // 4) * 4
    assert T0 * P <= 2 * WIN1, "layer-1 source exceeds two windows"
    NW1 = -(-(T0 * P) // WIN1)
    T1 = -(-(-(-SH // P)) // 4) * 4

    slot0 = []
    for c in range(NCORES):
        h = halos[c]
        o = np.argsort(-degi[h], kind="stable")
        sn = np.full(T0 * P, -1, np.int64)
        sn[:len(h)] = h[o]
        slot0.append(sn)
    degmat = np.zeros((NCORES, T0 * P), np.int64)
    for c in range(NCORES):
        v = slot0[c] >= 0
        degmat[c][v] = degi[slot0[c][v]]
    D0 = degmat.reshape(NCORES, T0, P).max(axis=(0, 2))  # [T0]
    D0t = tuple(int(v) for v in D0)
    D0Pt = tuple(-(-int(v) // 2) for v in D0)            # pair columns
    groups0, colbase0 = _schedule0(D0Pt)
    C0P = colbase0[T0]

    # layer-1 window vectors depend on each core's own slot0 (src slot ids)
    pos0s, wvec1s = [], []
    for c in range(NCORES):
        pos0 = np.full(N, -1, np.int64)
        v = slot0[c] >= 0
        pos0[slot0[c][v]] = np.nonzero(v)[0]
        pos0s.append(pos0)
        rep, _, srcs = _edge_expand(owns[c], degi, s_sorted, indptr)
        wv = np.zeros((SH, NW1), np.int64)
        np.add.at(wv, (rep, pos0[srcs] // WIN1), 1)
        wvec1s.append(wv)
    slot1 = [_assign_slots(owns[c], wvec1s[c], T1) for c in range(NCORES)]
    wvec1_full = []
    for c in range(NCORES):
        full = np.zeros((N, NW1), np.int64)
        full[owns[c]] = wvec1s[c]
        wvec1_full.append(full)
    D1 = np.max([_dprof_w(slot1[c], wvec1_full[c], T1, NW1)
                 for c in range(NCORES)], axis=0)
    D1t = tuple(tuple(int(v) for v in row) for row in D1)
    _, _, colbase1, C1 = _schedule(D1t, NW1)

    cb0 = np.asarray(colbase0[:T0], np.int64)
    cores = []
    for c in range(NCORES):
        s0, s1 = slot0[c], slot1[c]
        v0, v1 = s0 >= 0, s1 >= 0

        # layer-0 fused message table: transposed pair layout, fp8.
        # entry [(cb0[t]+c)*2+i, slot, f] = dis2[dst] * xhat[src of msg 2c+i]
        X0 = np.zeros((C0P * 2, P, IN), np.float32)
        k = np.nonzero(v0)[0]
        nodes = s0[k]
        rep, jj, srcs = _edge_expand(nodes, degi, s_sorted, indptr)
        slot = k[rep]
        col2 = cb0[slot // P] * 2 + jj
        X0[col2, slot % P] = xhat[srcs] * (S_T * dis2[nodes[rep]])[:, None]
        # -> [f, col2, slot] -> [P, C0P*2*P]
        x0 = _f8(X0.transpose(2, 0, 1).reshape(P, C0P * 2 * P))

        flat1, idx1 = _fill_idx(s1, D1t, colbase1, C1, degi, s_sorted, indptr,
                                pos0s[c], WIN1)

        def _scales(s, v, T, vec):
            iv = np.where(v, vec[np.clip(s, 0, None)], 0).astype(np.float32)
            return iv.reshape(1, T * P)

        inv0 = _scales(s0, v0, T0, dis)     # bias seed for layer 0 (non-ZB)
        inv1 = _scales(s1, v1, T1, invd)    # bias seed for layer 1 (non-ZB)
        # dis1 (dst-side sqrt-deg scale of layer 2) folded into mct
        mct = np.zeros((T1 * P, G), np.float32)
        mct[v1] = (Mhat[:, s1[v1]] * (dis[s1[v1]] / S_H2)[None, :]).T
        cores.append(dict(x0=x0, idx1=idx1,
                          inv0=inv0.astype(BF16), inv1=inv1.astype(BF16),
                          mct=mct.astype(BF16), flat1=flat1))

    w0d = np.concatenate([np.asarray(W0, np.float32)] * 2, axis=1)  # [IN,2*HID]
    shared = dict(
        w0d=_f8(S_W0 * w0d),
        w1=np.ascontiguousarray(W1, np.float32).reshape(2, P, HID).astype(BF16),
        b0r=(S_T * S_W0 * np.ascontiguousarray(b0, np.float32)
             ).reshape(1, HID).astype(BF16),
        b1r=np.ascontiguousarray(b1, np.float32).reshape(1, HID).astype(BF16),
        ident=np.eye(P, dtype=np.float32).astype(BF16),
    )
    zero_bias = bool(np.all(np.asarray(b0) == 0) and np.all(np.asarray(b1) == 0))
    meta = dict(N=N, IN=IN, HID=HID, G=G, SH=SH, T0=T0, T1=T1,
                NW1=NW1, C0P=C0P, C1=C1, zero_bias=zero_bias,
                D0P=D0Pt, D1=D1t)
    fin = dict(W2=np.asarray(W2, np.float32), b2=np.asarray(b2, np.float32))
    return meta, shared, cores, fin


# ---------------------------------------------------------------------------
# Pure-numpy emulation of the device program (validation / debugging)
# ---------------------------------------------------------------------------

def _emulate(meta, shared, cores, fin):
    T0, T1, HID, IN, G = (meta[k] for k in ("T0", "T1", "HID", "IN", "G"))
    NW1 = meta["NW1"]
    D0P, D1 = meta["D0P"], meta["D1"]
    _, colbase0 = _schedule0(D0P)
    C0P = colbase0[T0]
    _, calls1, colbase1, C1 = _schedule(D1, NW1)
    w0 = shared["w0d"].astype(np.float32)[:, :HID]
    w1 = shared["w1"].astype(np.float32).reshape(2 * P, HID)
    b0 = shared["b0r"].astype(np.float32)[0]
    b1 = shared["b1r"].astype(np.float32)[0]

    def col_windows(calls, C):
        cw = np.zeros(C, np.int64)
        for w, n, off in calls:
            cw[off:off + n] = w
        return cw

    cw1 = col_windows(calls1, C1)

    Y = np.zeros((G, HID), np.float32)
    for cd in cores:
        # table: [f, C0P*2, slot] -> [C0P*2, slot, f]
        tab = cd["x0"].astype(np.float32).reshape(P, C0P * 2, P)
        tab = tab.transpose(1, 2, 0)
        inv0 = cd["inv0"].astype(np.float32)[0]
        pre = inv0[:, None] * b0[None, :]
        pre = np.tile(pre.reshape(T0 * P, HID), (1, 1)).astype(np.float32)
        for t in range(T0):
            for c2 in range(D0P[t] * 2):
                pre[t * P:(t + 1) * P] += tab[colbase0[t] * 2 + c2] @ w0
        h1 = np.maximum(pre * ACT0_SCALE, 0).astype(BF16).astype(np.float32)
        # place h1 into windowed layout
        h1w = np.zeros((NW1 * WSTR, HID), np.float32)
        for w in range(NW1):
            sl0, sl1 = w * WIN1, min((w + 1) * WIN1, T0 * P)
            h1w[w * WSTR + 1: w * WSTR + 1 + (sl1 - sl0)] = h1[sl0:sl1]

        rows1 = cw1.repeat(P) * WSTR + cd["flat1"]
        gat1 = h1w[rows1].reshape(C1, P, HID)
        u1 = np.zeros((T1 * P, HID), np.float32)
        for t in range(T1):
            for w in range(NW1):
                cb = colbase1[t][w]
                for j in range(D1[t][w]):
                    u1[t * P:(t + 1) * P] += gat1[cb + j]
        u1 = u1.astype(BF16).astype(np.float32)
        inv1 = cd["inv1"].astype(np.float32)[0]
        pre1 = u1 @ w1 + inv1[:, None] * b1[None, :]
        h2 = np.clip(np.maximum(pre1 * ACT1_SCALE, 0), 0,
                     240).astype(F8).astype(np.float32)
        Y += cd["mct"].astype(np.float32).T @ h2
    return Y @ fin["W2"] + fin["b2"]


# ---------------------------------------------------------------------------
# Bass device program
# ---------------------------------------------------------------------------

def _build(meta):
    import concourse.bass as bass
    import concourse.mybir as mybir
    import concourse.tile as tile
    from concourse import bacc, library_config
    from concourse.tile_rust import add_dep_helper

    F32, I16 = mybir.dt.float32, mybir.dt.int16
    BF = mybir.dt.bfloat16
    F8D = mybir.dt.float8e4
    RELU = mybir.ActivationFunctionType.Relu
    DR = mybir.MatmulPerfMode.DoubleRow

    IN, HID, G = meta["IN"], meta["HID"], meta["G"]
    T0, T1 = meta["T0"], meta["T1"]
    NW1 = meta["NW1"]
    C0P, C1 = meta["C0P"], meta["C1"]
    D0P, D1 = meta["D0P"], meta["D1"]
    ZB = meta["zero_bias"]
    groups0, colbase0 = _schedule0(D0P)
    groups1, calls1, colbase1, _ = _schedule(D1, NW1)
    c2c1 = _call_of_col(calls1)

    nc = bacc.Bacc("TRN2", target_bir_lowering=False, debug=False,
                   num_devices=NCORES)

    t_x0 = nc.dram_tensor("x0", [P, C0P * 2 * P], F8D, kind="ExternalInput")
    t_idx1 = nc.dram_tensor("idx1", [P, C1 * 8], I16, kind="ExternalInput")
    t_inv0 = nc.dram_tensor("inv0", [1, T0 * P], BF, kind="ExternalInput")
    t_inv1 = nc.dram_tensor("inv1", [1, T1 * P], BF, kind="ExternalInput")
    t_w0d = nc.dram_tensor("w0d", [IN, 2 * HID], F8D, kind="ExternalInput")
    t_w1 = nc.dram_tensor("w1", [2, P, HID], BF, kind="ExternalInput")
    t_b0 = nc.dram_tensor("b0r", [1, HID], BF, kind="ExternalInput")
    t_b1 = nc.dram_tensor("b1r", [1, HID], BF, kind="ExternalInput")
    t_mct = nc.dram_tensor("mct", [T1 * P, G], BF, kind="ExternalInput")
    t_id = nc.dram_tensor("ident", [P, P], BF, kind="ExternalInput")
    t_out = nc.dram_tensor("outp", [P, 2, G], F32, kind="ExternalOutput")

    with tile.TileContext(nc) as tc:
        with (
            tc.tile_pool(name="const", bufs=1) as cpool,
            tc.tile_pool(name="ut", bufs=6) as upool,
            tc.tile_pool(name="stage", bufs=4) as spool,
            tc.tile_pool(name="h2sbp", bufs=1) as h2pool,
            tc.tile_pool(name="g1", bufs=5) as gpool1,
            tc.tile_pool(name="dram", bufs=1, space="DRAM") as dpool,
        ):
            lib = nc.gpsimd.load_library(library_config.mlp)

            def cload(t, shape, dt):
                s = cpool.tile(shape, dt, tag=t.name)
                nc.sync.dma_start(s[:], t[:])
                return s

            ident = cload(t_id, [P, P], BF)
            w0d = cload(t_w0d, [IN, 2 * HID], F8D)
            w1 = cpool.tile([P, 2, HID], BF, tag="w1")
            nc.sync.dma_start(w1[:], t_w1[:].rearrange("j p h -> p j h"))
            if not ZB:
                b0r = cload(t_b0, [1, HID], BF)
                b1r = cload(t_b1, [1, HID], BF)

            h1h = dpool.tile([NW1 * WSTR, HID], BF)
            h2sb = h2pool.tile([P, T1 * HID], F8D, tag="h2sb")

            # early loads on the Activation HWDGE queue: layer-1 indices +
            # pooling matrix
            idx1 = cpool.tile([P, C1 * 8], I16, tag="idx1")
            nc.scalar.dma_start(idx1[:], t_idx1[:])
            mct_all = cpool.tile([P, T1 * G], BF, tag="mct_all")
            nc.scalar.dma_start(
                mct_all[:].rearrange("p (t g) -> p t g", t=T1),
                t_mct[:].rearrange("(t p) g -> p t g", p=P))

            # h1h_writes[w]: writes a layer-1 gather from window w must wait on
            h1h_writes = [[] for _ in range(NW1)]
            zt = spool.tile([P, HID], BF, tag="zrow")
            nc.vector.memset(zt[:], 0.0)
            for w in range(NW1):
                h1h_writes[w].append(nc.scalar.dma_start(
                    h1h[w * WSTR:w * WSTR + 1, :], zt[0:1, :]))

            def slot_row(s):
                return (s // WIN1) * WSTR + 1 + (s % WIN1)

            # ---------------- Layer 0 ----------------
            with tc.tile_pool(name="x0p", bufs=3) as xpool, \
                 tc.tile_pool(name="pre0", bufs=2, space="PSUM") as ppool0:
                stage_t = None
                pre4 = None
                gi = 0
                xt = None
                off = 0
                for t in range(T0):
                    if gi < len(groups0) and t == groups0[gi][0]:
                        t0g, t1g = groups0[gi]
                        off = colbase0[t0g]
                        ncol = colbase0[t1g] - off
                        xt = xpool.tile([P, GC0 * 2 * P], F8D, tag="x0")
                        nc.sync.dma_start(
                            xt[:, :ncol * 2 * P],
                            t_x0[:, off * 2 * P:(off + ncol) * 2 * P])
                        gi += 1
                    q = t % 4
                    if q == 0:
                        pre4 = ppool0.tile([P, 4, 2 * HID], F32,
                                           tag="pre0", space="PSUM")
                    ncp = D0P[t]
                    base = colbase0[t] - off
                    if not ZB:
                        sd = upool.tile([1, P], BF, tag="seed")
                        nc.sync.dma_start(sd[:], t_inv0[:, t * P:(t + 1) * P])
                        nc.tensor.matmul(pre4[:, q, :HID], lhsT=sd[:],
                                         rhs=b0r[:], start=True, stop=False)
                    for c in range(ncp):
                        nc.tensor.matmul(
                            pre4[:, q, :HID],
                            lhsT=xt[:, (base + c) * 2 * P:(base + c + 1) * 2 * P
                                    ].rearrange("p (two m) -> p two m", two=2),
                            rhs=w0d[:].rearrange("p (two h) -> p two h", two=2),
                            start=(ZB and c == 0), stop=(c == ncp - 1),
                            perf_mode=DR)
                    if q == 3:
                        stage_t = spool.tile([P, 4 * HID], BF, tag="h1stage")
                        nc.scalar.activation(
                            stage_t[:].rearrange("p (j h) -> p j h", j=4),
                            pre4[:, :, :HID], RELU, bias=0.0,
                            scale=ACT0_SCALE)
                        s0r = (t - 3) * P
                        r0 = slot_row(s0r)
                        h1h_writes[s0r // WIN1].append(nc.scalar.dma_start(
                            h1h[r0:r0 + 4 * P, :].rearrange(
                                "(j p) h -> p j h", p=P),
                            stage_t[:].rearrange("p (j h) -> p j h", j=4)))

            # ---------------- Layer 1 + fused pool ----------------
            with tc.tile_pool(name="aggps", bufs=4, space="PSUM") as apool, \
                 tc.tile_pool(name="pre1", bufs=2, space="PSUM") as ppool1, \
                 tc.tile_pool(name="outps", bufs=1, space="PSUM") as opool:
                opsT0 = opool.tile([P, G], F32, tag="outps0", space="PSUM")
                opsT1 = opool.tile([P, G], F32, tag="outps1", space="PSUM")
                opsT = [opsT0, opsT1]

                def do_gathers(t_src_ap, idx_sb, calls, grp_calls, buf_pool,
                               ccap_elem, elem, deps_by_w):
                    out = {}
                    for ci in grp_calls:
                        w, ncols, off = calls[ci]
                        gt = buf_pool.tile([P, ccap_elem], BF, tag="g")
                        gi = nc.gpsimd.dma_gather(
                            gt[:, :ncols * elem].rearrange(
                                "p (j d) -> p j d", j=ncols),
                            t_src_ap(w),
                            idx_sb[:, off * 8:(off + ncols) * 8],
                            ncols * P, ncols * P, elem, single_packet=False)
                        add_dep_helper(gi.ins, lib.ins, True,
                                       "gather after lib")
                        for d in deps_by_w[w]:
                            add_dep_helper(gi.ins, d.ins, True,
                                           "gather after src")
                        out[ci] = gt
                    return out

                def grp_call_ids(calls, t0, t1, colbase, D, NW):
                    ids = set()
                    for tt in range(t0, t1):
                        for w in range(NW):
                            for j in range(D[tt][w]):
                                ids.add(c2c1[colbase[tt][w] + j][0])
                    return sorted(ids)

                def pool_mm(t):
                    for fh in range(2):
                        nc.tensor.matmul(
                            opsT[fh][:],
                            lhsT=h2sb[:, t * HID + fh * P:
                                      t * HID + (fh + 1) * P],
                            rhs=mct_all[:, t * G:(t + 1) * G],
                            start=(t == 0), stop=(t == T1 - 1))

                for (t0g, t1g) in groups1:
                    ids = grp_call_ids(calls1, t0g, t1g, colbase1, D1, NW1)
                    bufs = do_gathers(
                        lambda w: h1h[w * WSTR:(w + 1) * WSTR, :],
                        idx1, calls1, ids, gpool1, CCAP * HID, HID,
                        h1h_writes)
                    for t in range(t0g, t1g):
                        nd = sum(D1[t])
                        u0ps = apool.tile([P, P], F32, tag="aggps",
                                          space="PSUM")
                        u1ps = apool.tile([P, P], F32, tag="aggps",
                                          space="PSUM")
                        k = 0
                        for w in range(NW1):
                            for j in range(D1[t][w]):
                                ci, lc = c2c1[colbase1[t][w] + j]
                                gt = bufs[ci]
                                nc.tensor.matmul(
                                    u0ps[:],
                                    lhsT=gt[:, lc * HID:lc * HID + P],
                                    rhs=ident[:], start=(k == 0),
                                    stop=(k == nd - 1))
                                nc.tensor.matmul(
                                    u1ps[:],
                                    lhsT=gt[:, lc * HID + P:(lc + 1) * HID],
                                    rhs=ident[:], start=(k == 0),
                                    stop=(k == nd - 1))
                                k += 1
                        uT0 = upool.tile([P, P], BF, tag="ut")
                        uT1 = upool.tile([P, P], BF, tag="ut")
                        if nd == 0:
                            nc.vector.memset(uT0[:], 0.0)
                            nc.vector.memset(uT1[:], 0.0)
                        else:
                            nc.vector.tensor_copy(uT0[:], u0ps[:])
                            nc.vector.tensor_copy(uT1[:], u1ps[:])
                        pre1 = ppool1.tile([P, HID], F32, tag="pre1",
                                           space="PSUM")
                        if not ZB:
                            sd = upool.tile([1, P], BF, tag="seed")
                            nc.sync.dma_start(sd[:],
                                              t_inv1[:, t * P:(t + 1) * P])
                            nc.tensor.matmul(pre1[:], lhsT=sd[:],
                                             rhs=b1r[:], start=True,
                                             stop=False)
                        nc.tensor.matmul(pre1[:], lhsT=uT0[:],
                                         rhs=w1[:, 0, :],
                                         start=ZB, stop=False)
                        nc.tensor.matmul(pre1[:], lhsT=uT1[:],
                                         rhs=w1[:, 1, :],
                                         start=False, stop=True)
                        nc.scalar.activation(
                            h2sb[:, t * HID:(t + 1) * HID], pre1[:],
                            RELU, bias=0.0, scale=ACT1_SCALE)
                        pool_mm(t)

                osb = spool.tile([P, 2, G], F32, tag="osb")
                for fh in range(2):
                    nc.vector.tensor_copy(osb[:, fh, :], opsT[fh][:])
                nc.sync.dma_start(t_out[:], osb[:])

    nc.compile()
    return nc


# ---------------------------------------------------------------------------
# Entry point
# ---------------------------------------------------------------------------

_cache = {}


def _get_nc(meta):
    key = hashlib.sha1(repr(sorted(meta.items())).encode()).hexdigest()
    if key not in _cache:
        _cache[key] = _build(meta)
    return _cache[key]


def _in_maps(shared, cores):
    maps = []
    for cd in cores:
        m = dict(shared)
        m.update({k: cd[k] for k in
                  ("x0", "idx1", "inv0", "inv1", "mct")})
        maps.append(m)
    return maps


def _run_device(meta, shared, cores):
    from concourse.bass_utils import run_bass_kernel_spmd
    nc = _get_nc(meta)
    res = run_bass_kernel_spmd(nc, _in_maps(shared, cores),
                               core_ids=list(range(NCORES)))
    return [r["outp"] for r in res.results]


def kernel(**inputs):
    meta, shared, cores, fin = _preprocess(**inputs)
    outs = _run_device(meta, shared, cores)
    YT = np.sum(np.stack(outs), axis=0, dtype=np.float32)  # [P, 2, G]
    G = YT.shape[2]
    Y = YT.transpose(2, 1, 0).reshape(G, 2 * P)            # [G, HID]
    out = Y @ fin["W2"] + fin["b2"]
    return out.astype(np.float32)


def profile_run(meta, shared, cores, trace_cores=None):
    """Profiled exec time in ns: NTFF trace when available, else the
    instruction-cost-model timeline simulation of the compiled program."""
    from concourse.bass_utils import run_bass_kernel_spmd
    nc = _get_nc(meta)
    try:
        res = run_bass_kernel_spmd(nc, _in_maps(shared, cores),
                                   core_ids=list(range(NCORES)), trace=True,
                                   trace_cores=trace_cores)
        if res.exec_time_ns is not None:
            print("profile:", res.instructions_and_trace[1]
                  if res.instructions_and_trace else None)
            return res.exec_time_ns
    except Exception as e:
        print(f"NTFF trace unavailable ({type(e).__name__}); "
              "using cost-model timeline")
    from concourse.timeline_sim import TimelineSim
    ts = TimelineSim(nc, trace=False)
    ts.simulate()
    return int(ts.time)
